# revision 20
# baseline (speedup 1.0000x reference)
"""Trainium2 Bass kernel for AttentiveGraphPooling (gnn_message_passing).

Strategy: shard the 4096 graphs across 8 cores (512 graphs each). batch is
sorted, so each core owns a contiguous node range covering whole graphs ->
pooling / gather / GRU are all core-local, no collectives needed.

Per core, graphs go in 4 blocks of 128; nodes in NT tiles of 128 per block.
Resident per block: x node-major (bf16, for pooling), x feature-major in
fp8 DoubleRow layout (for the gate matmul), E^T one-hot (fp8, built once in
phase A via PE transpose).

Gate math: |w2| (and a 2^s scale) is folded into W1's columns host-side, so
  logit = sum_h sign(w2)_h * relu(h1'[n,h]),  h1' = x@W1'^T + GW1'[b(n)]
h1' is computed per node tile as ONE fp8 DoubleRow matmul (K=256) plus one
DoubleRow gather matmul (E^T tile paired with a zero k-tile). The whole
relu->*w2->reduce chain is ONE fused scalar_tensor_tensor (op0=max 0,
op1=mult sign-row, accum_out=logit column), split between the Vector and
GpSimd engines. Gates come from one batched sigmoid (scale=2^-s, bias=b2).

Weighted pooling stays bf16 (fp8 would lose too much precision in the
mean's cancellation): Eg one-hot built on DVE fused is_eq*gate, one matmul
per tile. Pool matmuls are emitted with a 2-batch lag so the PE runs long
uninterrupted bursts (p-state stays high).
"""

import os
import sys

import numpy as np

sys.path.insert(0, "/opt/trn_rl_repo")

H = 256
NBLK = 4  # graph blocks per core
GBLK = 128  # graphs per block
NUM_TIMESTEPS = 2
LCHUNK = 16  # node tiles per resident-load DMA
TB = 8  # gate batch (node tiles per sigmoid batch)
LAG = 2  # batches of lag for pool-matmul emission (PE burst length)
S_SCALE = 7  # 2^s fold into W1' so fp8 entries are in range

# fraction of fused logit ops on the vector engine (rest on gpsimd)
FUSED_DVE_NUM = 1
FUSED_DVE_DEN = 4


def _build_program(NT, nblk=NBLK):
    """Build the single-core SPMD Bass program. NT = node tiles per block."""
    from contextlib import ExitStack

    import concourse.bass as bass
    import concourse.tile as tile
    from concourse import bacc, mybir

    fp32 = mybir.dt.float32
    bf16 = mybir.dt.bfloat16
    fp8 = mybir.dt.float8e4
    DR = mybir.MatmulPerfMode.DoubleRow

    NTP = NT * 128  # padded nodes per block
    NB = NT // TB  # gate batches per block

    nc = bacc.Bacc("TRN2", target_bir_lowering=False, debug=False)

    # ---- DRAM parameters (per-core inputs) ----
    x_d = nc.dram_tensor("xk", [nblk * NTP, H], bf16, kind="ExternalInput")
    xt_d = nc.dram_tensor("xkT8", [nblk, 128, NT, 2, 128], fp8, kind="ExternalInput")
    bcols_d = nc.dram_tensor("bcols", [nblk, 128, NT], fp32, kind="ExternalInput")
    invc_d = nc.dram_tensor("invc", [nblk, GBLK, 1], fp32, kind="ExternalInput")
    w1dr_d = nc.dram_tensor("w1dr", [128, 2, H], fp8, kind="ExternalInput")
    w1tp_d = nc.dram_tensor("w1tp", [2, 128, H], bf16, kind="ExternalInput")
    b1p_d = nc.dram_tensor("b1prow", [1, H], bf16, kind="ExternalInput")
    sgn_d = nc.dram_tensor("sgnbc", [128, H], bf16, kind="ExternalInput")
    b2c_d = nc.dram_tensor("b2col", [128, 1], fp32, kind="ExternalInput")
    wih_d = nc.dram_tensor("wih_t", [2, 128, 3 * H], bf16, kind="ExternalInput")
    whh_d = nc.dram_tensor("whh_t", [2, 128, 3 * H], bf16, kind="ExternalInput")
    brz_d = nc.dram_tensor("bsum_rz", [1, 2 * H], fp32, kind="ExternalInput")
    bin_d = nc.dram_tensor("bihn", [1, H], fp32, kind="ExternalInput")
    bhn_d = nc.dram_tensor("bhhn", [1, H], fp32, kind="ExternalInput")
    iota_d = nc.dram_tensor("iota_row", [128, 128], bf16, kind="ExternalInput")
    eye_d = nc.dram_tensor("eye128", [128, 128], fp32, kind="ExternalInput")
    eyeb_d = nc.dram_tensor("eye128b", [128, 128], bf16, kind="ExternalInput")
    out_d = nc.dram_tensor("out", [nblk * GBLK, H], fp32, kind="ExternalOutput")

    with tile.TileContext(nc) as tc, ExitStack() as ctx:
        ep = ctx.enter_context  # shorthand

        const = ep(tc.tile_pool(name="const", bufs=1))
        xres = ep(tc.tile_pool(name="xres", bufs=3))
        xtres = ep(tc.tile_pool(name="xtres", bufs=2))
        etres = ep(tc.tile_pool(name="etres", bufs=2))
        bpool = ep(tc.tile_pool(name="bcols", bufs=2))
        epool = ep(tc.tile_pool(name="eoh", bufs=6))
        scr = ep(tc.tile_pool(name="scr", bufs=4))
        gtsb = ep(tc.tile_pool(name="gtsb", bufs=4))
        gsb = ep(tc.tile_pool(name="gsb", bufs=2))
        smallsb = ep(tc.tile_pool(name="smallsb", bufs=1))

        ps_pool = ep(tc.tile_pool(name="pspool", bufs=2, space="PSUM"))
        ps_h1 = ep(tc.tile_pool(name="psh1", bufs=4, space="PSUM"))
        ps_sm = ep(tc.tile_pool(name="pssm", bufs=2, space="PSUM"))

        # ---- load constants ----
        def cload(shape, src, tag, dt=fp32):
            t = const.tile(shape, dt, tag=tag)
            nc.sync.dma_start(t[:], src)
            return t

        iota_row = cload([128, 128], iota_d[:], "c_iota", bf16)
        eye = cload([128, 128], eye_d[:], "c_eye")
        eyeb = cload([128, 128], eyeb_d[:], "c_eyeb", bf16)
        w1dr = cload([128, 2, H], w1dr_d[:], "c_w1dr", fp8)
        w1tp = [cload([128, H], w1tp_d[k], f"c_w1tp{k}", bf16) for k in range(2)]
        b1p = cload([1, H], b1p_d[:], "c_b1p", bf16)
        sgnbc = cload([128, H], sgn_d[:], "c_sgn", bf16)
        b2col = cload([128, 1], b2c_d[:], "c_b2c")
        wih = [cload([128, 3 * H], wih_d[k], f"c_wih{k}", bf16) for k in range(2)]
        whh = [cload([128, 3 * H], whh_d[k], f"c_whh{k}", bf16) for k in range(2)]
        brz = cload([1, 2 * H], brz_d[:], "c_brz")
        bin_ = cload([1, H], bin_d[:], "c_bin")
        bhn = cload([1, H], bhn_d[:], "c_bhn")
        invc = [cload([GBLK, 1], invc_d[j], f"c_invc{j}") for j in range(nblk)]
        ones_row = const.tile([1, 128], fp32)
        nc.vector.memset(ones_row[:], 1.0)
        ones_bf = const.tile([1, 128], bf16)
        nc.vector.memset(ones_bf[:], 1.0)

        def fm_copy(g_ap, pool, tag, dt):
            """(128,256) graph-major -> feature-major (128,2,128) via PE."""
            gf = pool.tile([128, 2, GBLK], dt, tag=tag)
            tp = ps_sm.tile([128, 2, 128], fp32, tag="pssm")
            for ki in range(2):
                nc.tensor.matmul(tp[:, ki, :], g_ap[:, ki * 128 : (ki + 1) * 128],
                                 eye[:], is_transpose=True, start=True, stop=True)
                nc.scalar.copy(gf[:, ki, :], tp[:, ki, :])
            return gf

        NH = NT // 2  # resident x split in halves so the next block's first
        # half can load while this block's second half is still in use

        def fused_logit(h1p_ap, scratch_ap, acc_ap):
            """accum(relu(h1) * sign-row); DVE only (GPSIMD can't read PSUM)."""
            nc.vector.scalar_tensor_tensor(
                scratch_ap, h1p_ap, 0.0, sgnbc[:],
                op0=mybir.AluOpType.max, op1=mybir.AluOpType.mult,
                accum_out=acc_ap,
            )

        for j in range(nblk):
            # bcols first: it unblocks all one-hot builds
            bt = bpool.tile([128, NT], fp32, tag="bcols")
            nc.sync.dma_start(bt[:], bcols_d[j])

            # ---- resident x (node-major bf16) for this block, two halves ----
            xhalves = []
            for h0 in (0, NH):
                xh = xres.tile([128, NH, H], bf16, tag="xres")
                for c0 in range(0, NH, LCHUNK):
                    cn = min(LCHUNK, NH - c0)
                    base = j * NTP + (h0 + c0) * 128
                    src = x_d[base : base + cn * 128, :].rearrange(
                        "(c p) h -> p c h", p=128
                    )
                    nc.sync.dma_start(xh[:, c0 : c0 + cn, :], src)
                xhalves.append(xh)

            def xat(t):
                return xhalves[t // NH][:, t % NH, :]

            # ---- resident x^T fp8 DoubleRow layout, two halves ----
            xthalves = []
            for h0 in (0, NH):
                xth = xtres.tile([128, NH, 2, 128], fp8, tag="xtres")
                for c0 in range(0, NH, LCHUNK):
                    cn = min(LCHUNK, NH - c0)
                    nc.sync.dma_start(
                        xth[:, c0 : c0 + cn, :, :],
                        xt_d[j, :, h0 + c0 : h0 + c0 + cn, :, :],
                    )
                xthalves.append(xth)

            def xtat(t):
                return xthalves[t // NH][:, t % NH, :, :]

            etj = etres.tile([128, NT + 1, 128], fp8, tag="etres")
            nc.gpsimd.memset(etj[:, NT, :], 0.0)

            # ---- phase A: initial mean pool + resident E^T build ----
            # e-builds run ahead (DVE); transposes+pool mms lag so the PE
            # sees contiguous work.
            pooled = ps_pool.tile([GBLK, H], fp32, tag="pspool")
            ephA = []
            for t in range(NT + LAG):
                if t < NT:
                    e = epool.tile([128, 128], bf16, tag="eoh")
                    nc.gpsimd.tensor_scalar(
                        e[:], iota_row[:], bt[:, t : t + 1], None,
                        op0=mybir.AluOpType.is_equal,
                    )
                    ephA.append(e)
                tl = t - LAG
                if tl >= 0:
                    e = ephA[tl]
                    nc.tensor.matmul(
                        pooled[:], e[:], xat(tl), start=(tl == 0),
                        stop=(tl == NT - 1), skip_group_check=True,
                    )
                    tp = ps_sm.tile([128, 128], bf16, tag="pssm")
                    nc.tensor.matmul(tp[:], e[:], eyeb[:], is_transpose=True,
                                     start=True, stop=True)
                    nc.scalar.copy(etj[:, tl, :], tp[:])
            g_gm = gsb.tile([GBLK, H], fp32, tag="gsb")
            nc.vector.tensor_scalar(
                g_gm[:], pooled[:], invc[j][:], None, op0=mybir.AluOpType.mult
            )
            g_fm = fm_copy(g_gm[:], gsb, "gfm", bf16)

            # ---- timesteps ----
            for ts in range(NUM_TIMESTEPS):
                # GW1' = G @ W1'^T + b1'  (graph-level, bf16 -> fp8 copy)
                gw1p = ps_sm.tile([GBLK, H], fp32, tag="pssm")
                for ki in range(2):
                    nc.tensor.matmul(gw1p[:], g_fm[:, ki, :], w1tp[ki][:],
                                     start=(ki == 0), stop=False,
                                     skip_group_check=True)
                nc.tensor.matmul(gw1p[:], ones_bf[:], b1p[:],
                                 start=False, stop=True, skip_group_check=True)
                # fp8 copy padded with a zero k-tile for DoubleRow gather
                gw18z = gsb.tile([GBLK, 2, H], fp8, tag="gw18z")
                nc.gpsimd.memset(gw18z[:, 1, :], 0.0)
                nc.scalar.copy(gw18z[:, 0, :], gw1p[:])

                pooled = ps_pool.tile([GBLK, H], fp32, tag="pspool")

                # software-pipelined batches: h1 mms + fused logit run
                # ahead; sigmoid, eg builds and pool matmuls lag.
                h1ps = {}
                gacc = {}
                gt = {}

                def emit_h1(b):
                    ga = gtsb.tile([128, TB], fp32, tag="gacc")
                    gacc[b] = ga
                    for c in range(TB):
                        t = b * TB + c
                        if c % 2 == 0:
                            h1pair = ps_h1.tile([128, 2, H], fp32, tag="psh1")
                        h1ps[t] = h1pair[:, c % 2, :]
                        nc.tensor.matmul(h1ps[t], xtat(t), w1dr[:],
                                         start=True, stop=False, perf_mode=DR)
                        nc.tensor.matmul(h1ps[t], etj[:, t : t + 2, :],
                                         gw18z[:], start=False, stop=True,
                                         perf_mode=DR)

                def emit_fused(b):
                    for c in range(TB):
                        t = b * TB + c
                        sc = scr.tile([128, H], bf16, tag="scr")
                        fused_logit(h1ps[t], sc[:],
                                    gacc[b][:, c : c + 1])
                        del h1ps[t]

                def emit_gate_pool(b):
                    g = gtsb.tile([128, TB], fp32, tag="gt")
                    gt[b] = g
                    nc.scalar.activation(
                        g[:], gacc[b][:],
                        mybir.ActivationFunctionType.Sigmoid,
                        bias=b2col[:], scale=float(2.0 ** (-S_SCALE)),
                    )
                    for c in range(TB):
                        t = b * TB + c
                        eg = epool.tile([128, 128], bf16, tag="eoh")
                        nc.gpsimd.tensor_scalar(
                            eg[:], iota_row[:], bt[:, t : t + 1],
                            g[:, c : c + 1],
                            op0=mybir.AluOpType.is_equal,
                            op1=mybir.AluOpType.mult,
                        )
                        nc.tensor.matmul(
                            pooled[:], eg[:], xat(t),
                            start=(t == 0), stop=(t == NT - 1),
                            skip_group_check=True,
                        )

                for b in range(NB + LAG):
                    if b < NB:
                        emit_h1(b)
                        emit_fused(b)
                    bl = b - LAG
                    if bl >= 0:
                        emit_gate_pool(bl)

                ps = gsb.tile([GBLK, H], fp32, tag="poolsb")
                nc.vector.tensor_scalar(
                    ps[:], pooled[:], invc[j][:], None,
                    op0=mybir.AluOpType.mult
                )
                pf = fm_copy(ps[:], gsb, "poolfm", bf16)

                # ---- GRU cell (graph-major) ----
                gf, h_old = g_fm, g_gm

                def gru_mm(psum, wi, wh, bias_row, bcol0, bn):
                    mms = []
                    if wi is not None:
                        mms += [(pf[:, ki, :], wi[ki][:, bcol0 : bcol0 + bn])
                                for ki in range(2)]
                    if wh is not None:
                        mms += [(gf[:, ki, :], wh[ki][:, bcol0 : bcol0 + bn])
                                for ki in range(2)]
                    for i, (lhsT, rhs) in enumerate(mms):
                        nc.tensor.matmul(
                            psum[:], lhsT, rhs, start=(i == 0), stop=False,
                            skip_group_check=True,
                        )
                    nc.tensor.matmul(
                        psum[:], ones_row[:], bias_row, start=False, stop=True,
                        skip_group_check=True,
                    )

                rp = ps_h1.tile([GBLK, H], fp32, tag="psh1")
                gru_mm(rp, wih, whh, brz[:, 0:H], 0, H)
                r = smallsb.tile([GBLK, H], fp32, tag="gru_r")
                nc.scalar.activation(r[:], rp[:],
                                     mybir.ActivationFunctionType.Sigmoid)
                zp = ps_h1.tile([GBLK, H], fp32, tag="psh1")
                gru_mm(zp, wih, whh, brz[:, H : 2 * H], H, H)
                z = smallsb.tile([GBLK, H], fp32, tag="gru_z")
                nc.scalar.activation(z[:], zp[:],
                                     mybir.ActivationFunctionType.Sigmoid)
                inp_ = ps_h1.tile([GBLK, H], fp32, tag="psh1")
                gru_mm(inp_, wih, None, bin_[:], 2 * H, H)
                hnp = ps_h1.tile([GBLK, H], fp32, tag="psh1")
                gru_mm(hnp, None, whh, bhn[:], 2 * H, H)
                t1 = smallsb.tile([GBLK, H], fp32, tag="gru_s1")
                nc.vector.tensor_mul(t1[:], r[:], hnp[:])
                t2 = smallsb.tile([GBLK, H], fp32, tag="gru_s2")
                nc.vector.tensor_add(t2[:], t1[:], inp_[:])
                n = smallsb.tile([GBLK, H], fp32, tag="gru_n")
                nc.scalar.activation(n[:], t2[:],
                                     mybir.ActivationFunctionType.Tanh)
                t3 = smallsb.tile([GBLK, H], fp32, tag="gru_s1")
                nc.vector.tensor_sub(t3[:], h_old[:], n[:])
                t4 = smallsb.tile([GBLK, H], fp32, tag="gru_s2")
                nc.vector.tensor_mul(t4[:], z[:], t3[:])
                t5 = smallsb.tile([GBLK, H], fp32, tag="gru_s3")
                nc.vector.tensor_add(t5[:], n[:], t4[:])
                g_gm = gsb.tile([GBLK, H], fp32, tag="gsb")
                nc.scalar.activation(g_gm[:], t5[:],
                                     mybir.ActivationFunctionType.Relu)
                if ts < NUM_TIMESTEPS - 1:
                    g_fm = fm_copy(g_gm[:], gsb, "gfm", bf16)

            nc.sync.dma_start(out_d[j * GBLK : (j + 1) * GBLK, :], g_gm[:])

    nc.compile()
    return nc


def _prep_inputs(x, batch, counts, n_cores, nblk, NT=None):
    """Host-side shard + pad + layout. Returns (per_core, NT)."""
    import ml_dtypes

    G = n_cores * nblk * GBLK
    batch = np.asarray(batch).astype(np.int64)
    x = np.asarray(x, dtype=np.float32)

    edges = np.searchsorted(batch, np.arange(0, G + 1, GBLK))
    blk_cnt = np.diff(edges)
    if NT is None:
        NT = int(np.ceil(blk_cnt.max() / 128))
        NT = ((NT + TB - 1) // TB) * TB
        if (NT // TB) % 2:
            NT += TB  # halves must hold whole TB batches
    NTP = NT * 128

    invc_all = (1.0 / np.maximum(counts, 1.0)).astype(np.float32)

    xb = x.astype(ml_dtypes.bfloat16)
    x8 = x.astype(ml_dtypes.float8_e4m3)
    per_core = []
    for k in range(n_cores):
        xk = np.zeros((nblk * NTP, H), dtype=ml_dtypes.bfloat16)
        xk8 = np.zeros((nblk, NTP, H), dtype=ml_dtypes.float8_e4m3)
        bcols = np.full((nblk, 128, NT), -1.0, dtype=np.float32)
        for j in range(nblk):
            bi = k * nblk + j
            lo, hi = edges[bi], edges[bi + 1]
            cnt = hi - lo
            xk[j * NTP : j * NTP + cnt] = xb[lo:hi]
            xk8[j, :cnt] = x8[lo:hi]
            blp = np.full(NTP, -1.0, dtype=np.float32)
            blp[:cnt] = (batch[lo:hi] - (bi * GBLK)).astype(np.float32)
            bcols[j] = blp.reshape(NT, 128).T
        # [j, p(feat in ktile), t, k(ktile), n] = x8[j, t*128+n, k*128+p]
        xkT8 = np.ascontiguousarray(
            xk8.reshape(nblk, NT, 128, 2, 128).transpose(0, 4, 1, 3, 2)
        )
        invc = invc_all[k * nblk * GBLK : (k + 1) * nblk * GBLK].reshape(
            nblk, GBLK, 1
        )
        per_core.append({"xk": xk, "xkT8": xkT8, "bcols": bcols,
                         "invc": np.ascontiguousarray(invc)})
    return per_core, NT


def _const_inputs(gate_w1, gate_b1, gate_w2, gate_b2, gru_w_ih, gru_w_hh,
                  gru_b_ih, gru_b_hh):
    import ml_dtypes

    f = np.float32
    bf = ml_dtypes.bfloat16
    f8 = ml_dtypes.float8_e4m3
    c = {}
    w1 = np.asarray(gate_w1, f)  # (H, H), h1 = x @ w1.T
    w2 = np.asarray(gate_w2, f).reshape(H)
    scale = (2.0 ** S_SCALE)
    w1p = w1 * (np.abs(w2)[:, None] * scale)  # fold |w2| into rows of W1
    b1p = np.asarray(gate_b1, f) * np.abs(w2) * scale
    sgn = np.where(w2 >= 0.0, 1.0, -1.0).astype(f)
    # w1dr[p, k, h] = w1p[h, k*128+p]
    c["w1dr"] = np.ascontiguousarray(
        w1p.T.reshape(2, 128, H).transpose(1, 0, 2)).astype(f8)
    c["w1tp"] = np.ascontiguousarray(w1p.T.reshape(2, 128, H)).astype(bf)
    c["b1prow"] = b1p.reshape(1, H).astype(bf)
    c["sgnbc"] = np.tile(sgn.reshape(1, H), (128, 1)).astype(bf)
    c["b2col"] = np.full((128, 1), np.asarray(gate_b2, f).reshape(()), dtype=f)
    c["wih_t"] = np.ascontiguousarray(
        np.asarray(gru_w_ih, f).T).reshape(2, 128, 3 * H).astype(bf)
    c["whh_t"] = np.ascontiguousarray(
        np.asarray(gru_w_hh, f).T).reshape(2, 128, 3 * H).astype(bf)
    bih = np.asarray(gru_b_ih, f)
    bhh = np.asarray(gru_b_hh, f)
    c["bsum_rz"] = (bih[: 2 * H] + bhh[: 2 * H]).reshape(1, 2 * H)
    c["bihn"] = bih[2 * H :].reshape(1, H)
    c["bhhn"] = bhh[2 * H :].reshape(1, H)
    c["iota_row"] = np.tile(np.arange(128, dtype=f), (128, 1)).astype(bf)
    c["eye128"] = np.eye(128, dtype=f)
    c["eye128b"] = np.eye(128, dtype=f).astype(bf)
    return c


_CACHE = {}


def run(x, gate_w1, gate_b1, gate_w2, gate_b2, gru_w_ih, gru_w_hh, gru_b_ih,
        gru_b_hh, batch, num_graphs, n_cores=8, nblk=NBLK, trace=False,
        use_sim=False):
    from concourse.bass_utils import run_bass_kernel_spmd

    batch = np.asarray(batch).astype(np.int64)
    G = n_cores * nblk * GBLK
    counts = np.bincount(batch, minlength=G).astype(np.float32)
    per_core, NT = _prep_inputs(x, batch, counts, n_cores, nblk)
    consts = _const_inputs(gate_w1, gate_b1, gate_w2, gate_b2, gru_w_ih,
                           gru_w_hh, gru_b_ih, gru_b_hh)
    in_maps = [{**consts, **pc} for pc in per_core]

    key = (NT, nblk, n_cores)
    if key not in _CACHE:
        _CACHE[key] = _build_program(NT, nblk=nblk)
    nc = _CACHE[key]

    if use_sim:
        from concourse.bass_interp import CoreSim

        outs = []
        for k in range(n_cores):
            sim = CoreSim(nc)
            for name, arr in in_maps[k].items():
                sim.tensor(name)[:] = arr
            sim.simulate()
            outs.append(np.array(sim.tensor("out")))
        return np.concatenate(outs, axis=0), None

    res = run_bass_kernel_spmd(nc, in_maps, core_ids=list(range(n_cores)),
                               trace=trace)
    out = np.concatenate([res.results[k]["out"] for k in range(n_cores)], axis=0)
    return out, res


def kernel(**inputs):
    out, _ = run(**inputs)
    return out


# revision 36
# speedup vs baseline: 2.0620x; 2.0620x over previous
"""Trainium2 Bass kernel for AttentiveGraphPooling (gnn_message_passing).

Strategy: shard the 4096 graphs across 8 cores (512 graphs each). batch is
sorted, so each core owns a contiguous node range covering whole graphs ->
pooling / gather / GRU are all core-local, no collectives needed.

Per core, graphs go in 4 blocks of 128; nodes in NT tiles of 128 per block.
Resident per block: x node-major (bf16, for pooling), x feature-major in
fp8 DoubleRow layout (for the gate matmul), E^T one-hot (fp8, built once in
phase A via PE transpose).

Gate math: |w2| (and a 2^s scale) is folded into W1's columns host-side, so
  logit = sum_h sign(w2)_h * relu(h1'[n,h]),  h1' = x@W1'^T + GW1'[b(n)]
h1' is computed per node tile as ONE fp8 DoubleRow matmul (K=256) plus one
DoubleRow gather matmul (E^T tile paired with a zero k-tile). The whole
relu->*w2->reduce chain is ONE fused scalar_tensor_tensor (op0=max 0,
op1=mult sign-row, accum_out=logit column), split between the Vector and
GpSimd engines. Gates come from one batched sigmoid (scale=2^-s, bias=b2).

Weighted pooling stays bf16 (fp8 would lose too much precision in the
mean's cancellation): Eg one-hot built on DVE fused is_eq*gate, one matmul
per tile. Pool matmuls are emitted with a 2-batch lag so the PE runs long
uninterrupted bursts (p-state stays high).
"""

import os
import sys

import numpy as np

sys.path.insert(0, "/opt/trn_rl_repo")

H = 256
NBLK = 4  # graph blocks per core
GBLK = 128  # graphs per block
NUM_TIMESTEPS = 2
LCHUNK = 16  # node tiles per resident-load DMA
TB = 8  # gate batch (node tiles per sigmoid batch)
LAG = 2  # batches of lag for pool-matmul emission (PE burst length)
S_SCALE = 7  # 2^s fold into W1' so fp8 entries are in range

# fraction of gate batches drained by the Scalar engine (sign-split relu
# accumulate); the rest use the DVE fused op
ACT_NUM = 2
ACT_DEN = 5


def _build_program(NT, npos, nblk=NBLK):
    """Build the single-core SPMD Bass program. NT = node tiles per block."""
    from contextlib import ExitStack

    import concourse.bass as bass
    import concourse.tile as tile
    from concourse import bacc, mybir

    fp32 = mybir.dt.float32
    bf16 = mybir.dt.bfloat16
    fp8 = mybir.dt.float8e4
    DR = mybir.MatmulPerfMode.DoubleRow

    NTP = NT * 128  # padded nodes per block
    NB = NT // TB  # gate batches per block

    nc = bacc.Bacc("TRN2", target_bir_lowering=False, debug=False)

    # ---- DRAM parameters (per-core inputs) ----
    assert 0 < npos < H
    x_d = nc.dram_tensor("xk", [nblk * NTP, H], bf16, kind="ExternalInput")
    xt_d = nc.dram_tensor("xkT8", [nblk, 128, NT, 2, 128], fp8, kind="ExternalInput")
    bcols_d = nc.dram_tensor("bcols", [nblk, 128, NT], fp32, kind="ExternalInput")
    invc_d = nc.dram_tensor("invc", [nblk, GBLK, 1], fp32, kind="ExternalInput")
    w1dr_d = nc.dram_tensor("w1dr", [128, 2, H], fp8, kind="ExternalInput")
    w1tp_d = nc.dram_tensor("w1tp", [2, 128, H], bf16, kind="ExternalInput")
    b1p_d = nc.dram_tensor("b1prow", [1, H], bf16, kind="ExternalInput")
    sgn_d = nc.dram_tensor("sgnbc", [128, H], bf16, kind="ExternalInput")
    b2c_d = nc.dram_tensor("b2col", [128, 1], fp32, kind="ExternalInput")
    wih_d = nc.dram_tensor("wih_t", [2, 128, 3 * H], bf16, kind="ExternalInput")
    whh_d = nc.dram_tensor("whh_t", [2, 128, 3 * H], bf16, kind="ExternalInput")
    brz_d = nc.dram_tensor("bsum_rz", [1, 2 * H], bf16, kind="ExternalInput")
    bin_d = nc.dram_tensor("bihn", [1, H], bf16, kind="ExternalInput")
    bhn_d = nc.dram_tensor("bhhn", [1, H], bf16, kind="ExternalInput")
    iota_d = nc.dram_tensor("iota_row", [128, 128], bf16, kind="ExternalInput")
    eye_d = nc.dram_tensor("eye128", [128, 128], fp32, kind="ExternalInput")
    eyeb_d = nc.dram_tensor("eye128b", [128, 128], bf16, kind="ExternalInput")
    out_d = nc.dram_tensor("out", [nblk * GBLK, H], fp32, kind="ExternalOutput")

    with tile.TileContext(nc) as tc, ExitStack() as ctx:
        ep = ctx.enter_context  # shorthand

        const = ep(tc.tile_pool(name="const", bufs=1))
        xres = ep(tc.tile_pool(name="xres", bufs=3))
        xtres = ep(tc.tile_pool(name="xtres", bufs=2))
        etres = ep(tc.tile_pool(name="etres", bufs=1))
        bpool = ep(tc.tile_pool(name="bcols", bufs=2))
        epool = ep(tc.tile_pool(name="eoh", bufs=6))
        scr = ep(tc.tile_pool(name="scr", bufs=4))
        gtsb = ep(tc.tile_pool(name="gtsb", bufs=4))
        gsb = ep(tc.tile_pool(name="gsb", bufs=2))
        smallsb = ep(tc.tile_pool(name="smallsb", bufs=1))

        ps_pool = ep(tc.tile_pool(name="pspool", bufs=2, space="PSUM"))
        ps_h1 = ep(tc.tile_pool(name="psh1", bufs=4, space="PSUM"))
        ps_sm = ep(tc.tile_pool(name="pssm", bufs=2, space="PSUM"))

        # ---- load constants ----
        def cload(shape, src, tag, dt=fp32):
            t = const.tile(shape, dt, tag=tag)
            nc.sync.dma_start(t[:], src)
            return t

        iota_row = cload([128, 128], iota_d[:], "c_iota", bf16)
        eye = cload([128, 128], eye_d[:], "c_eye")
        eyeb = cload([128, 128], eyeb_d[:], "c_eyeb", bf16)
        w1dr = cload([128, 2, H], w1dr_d[:], "c_w1dr", fp8)
        w1tp = [cload([128, H], w1tp_d[k], f"c_w1tp{k}", bf16) for k in range(2)]
        b1p = cload([1, H], b1p_d[:], "c_b1p", bf16)
        sgnbc = cload([128, H], sgn_d[:], "c_sgn", bf16)
        b2col = cload([128, 1], b2c_d[:], "c_b2c")
        wih = [cload([128, 3 * H], wih_d[k], f"c_wih{k}", bf16) for k in range(2)]
        whh = [cload([128, 3 * H], whh_d[k], f"c_whh{k}", bf16) for k in range(2)]
        brz = cload([1, 2 * H], brz_d[:], "c_brz", bf16)
        bin_ = cload([1, H], bin_d[:], "c_bin", bf16)
        bhn = cload([1, H], bhn_d[:], "c_bhn", bf16)
        invc = [cload([GBLK, 1], invc_d[j], f"c_invc{j}") for j in range(nblk)]
        ones_row = const.tile([1, 128], fp32)
        nc.vector.memset(ones_row[:], 1.0)
        ones_bf = const.tile([1, 128], bf16)
        nc.vector.memset(ones_bf[:], 1.0)

        def fm_copy(g_ap, pool, tag, dt):
            """(128,256) graph-major -> feature-major (128,2,128) via PE."""
            gf = pool.tile([128, 2, GBLK], dt, tag=tag)
            tp = ps_sm.tile([128, 2, 128], fp32, tag="pssm")
            for ki in range(2):
                nc.tensor.matmul(tp[:, ki, :], g_ap[:, ki * 128 : (ki + 1) * 128],
                                 eye[:], is_transpose=True, start=True, stop=True)
                nc.scalar.copy(gf[:, ki, :], tp[:, ki, :])
            return gf

        NH = NT // 2  # resident x split in halves so the next block's first
        # half can load while this block's second half is still in use

        def fused_logit(h1p_ap, scratch_ap, acc_ap):
            """accum(relu(h1) * sign-row); DVE only (GPSIMD can't read PSUM)."""
            nc.vector.scalar_tensor_tensor(
                scratch_ap, h1p_ap, 0.0, sgnbc[:],
                op0=mybir.AluOpType.max, op1=mybir.AluOpType.mult,
                accum_out=acc_ap,
            )

        for j in range(nblk):
            # bcols first: it unblocks all one-hot builds
            bt = bpool.tile([128, NT], fp32, tag="bcols")
            nc.sync.dma_start(bt[:], bcols_d[j])

            # ---- resident x (node-major bf16) for this block, two halves ----
            xhalves = []
            for h0 in (0, NH):
                xh = xres.tile([128, NH, H], bf16, tag="xres")
                for c0 in range(0, NH, LCHUNK):
                    cn = min(LCHUNK, NH - c0)
                    base = j * NTP + (h0 + c0) * 128
                    src = x_d[base : base + cn * 128, :].rearrange(
                        "(c p) h -> p c h", p=128
                    )
                    nc.sync.dma_start(xh[:, c0 : c0 + cn, :], src)
                xhalves.append(xh)

            def xat(t):
                return xhalves[t // NH][:, t % NH, :]

            # ---- resident x^T fp8 DoubleRow layout, two halves ----
            xthalves = []
            for h0 in (0, NH):
                xth = xtres.tile([128, NH, 2, 128], fp8, tag="xtres")
                for c0 in range(0, NH, LCHUNK):
                    cn = min(LCHUNK, NH - c0)
                    nc.sync.dma_start(
                        xth[:, c0 : c0 + cn, :, :],
                        xt_d[j, :, h0 + c0 : h0 + c0 + cn, :, :],
                    )
                xthalves.append(xth)

            def xtat(t):
                return xthalves[t // NH][:, t % NH, :, :]

            etj = etres.tile([128, NT, 128], bf16, tag="etres")

            # ---- phase A: initial mean pool + resident E^T build ----
            # e-builds run ahead (DVE); E^T via XBAR DMA transpose; pool
            # matmuls lag so the PE sees contiguous work.
            pooled = ps_pool.tile([GBLK, H], fp32, tag="pspool")
            ephA = []
            for t in range(NT + LAG):
                if t < NT:
                    e = epool.tile([128, 128], bf16, tag="eoh")
                    nc.vector.tensor_scalar(
                        e[:], iota_row[:], bt[:, t : t + 1], None,
                        op0=mybir.AluOpType.is_equal,
                    )
                    nc.sync.dma_start_transpose(out=etj[:, t, :], in_=e[:])
                    ephA.append(e)
                tl = t - LAG
                if tl >= 0:
                    e = ephA[tl]
                    nc.tensor.matmul(
                        pooled[:], e[:], xat(tl), start=(tl == 0),
                        stop=(tl == NT - 1), skip_group_check=True,
                    )
            g_gm = gsb.tile([GBLK, H], fp32, tag="gsb")
            nc.vector.tensor_scalar(
                g_gm[:], pooled[:], invc[j][:], None, op0=mybir.AluOpType.mult
            )
            g_fm = fm_copy(g_gm[:], gsb, "gfm", bf16)

            # ---- timesteps ----
            for ts in range(NUM_TIMESTEPS):
                # GW1' = G @ W1'^T + b1'  (graph-level, bf16 -> fp8 copy)
                gw1p = ps_sm.tile([GBLK, H], fp32, tag="pssm")
                for ki in range(2):
                    nc.tensor.matmul(gw1p[:], g_fm[:, ki, :], w1tp[ki][:],
                                     start=(ki == 0), stop=False,
                                     skip_group_check=True)
                nc.tensor.matmul(gw1p[:], ones_bf[:], b1p[:],
                                 start=False, stop=True, skip_group_check=True)
                gw1bf = gsb.tile([GBLK, H], bf16, tag="gw1bf")
                nc.scalar.copy(gw1bf[:], gw1p[:])

                pooled = ps_pool.tile([GBLK, H], fp32, tag="pspool")

                # software-pipelined batches: h1 mms + fused logit run
                # ahead; sigmoid, eg builds and pool matmuls lag.
                h1ps = {}
                gacc = {}
                gt = {}

                def is_act_batch(b):
                    return (b % ACT_DEN) < ACT_NUM

                def emit_h1(b):
                    gaP = gtsb.tile([128, TB], fp32, tag="gacc", name="gaP")
                    gaN = (gtsb.tile([128, TB], fp32, tag="gaccN", name="gaN")
                           if is_act_batch(b) else None)
                    gacc[b] = (gaP, gaN)
                    for c in range(TB):
                        t = b * TB + c
                        if c % 2 == 0:
                            h1pair = ps_h1.tile([128, 2, H], fp32, tag="psh1")
                        h1ps[t] = h1pair[:, c % 2, :]
                        nc.tensor.matmul(h1ps[t], xtat(t), w1dr[:],
                                         start=True, stop=False, perf_mode=DR)
                        nc.tensor.matmul(h1ps[t], etj[:, t, :], gw1bf[:],
                                         start=False, stop=True)

                def emit_fused(b):
                    gaP, gaN = gacc[b]
                    for c in range(TB):
                        t = b * TB + c
                        if gaN is not None:
                            # sign-split relu accumulate on the Scalar engine
                            scp = scr.tile([128, npos], bf16, tag="scrp")
                            nc.scalar.activation(
                                scp[:], h1ps[t][:, :npos],
                                mybir.ActivationFunctionType.Relu,
                                accum_out=gaP[:, c : c + 1],
                            )
                            scn = scr.tile([128, H - npos], bf16, tag="scrn")
                            nc.scalar.activation(
                                scn[:], h1ps[t][:, npos:],
                                mybir.ActivationFunctionType.Relu,
                                accum_out=gaN[:, c : c + 1],
                            )
                        else:
                            sc = scr.tile([128, H], bf16, tag="scr")
                            fused_logit(h1ps[t], sc[:], gaP[:, c : c + 1])
                        del h1ps[t]

                def emit_gate_pool(b):
                    gaP, gaN = gacc[b]
                    if gaN is not None:
                        gd = gtsb.tile([128, TB], fp32, tag="gacc")
                        nc.vector.tensor_sub(gd[:], gaP[:], gaN[:])
                        gaP = gd
                    g = gtsb.tile([128, TB], fp32, tag="gt")
                    gt[b] = g
                    nc.scalar.activation(
                        g[:], gaP[:],
                        mybir.ActivationFunctionType.Sigmoid,
                        bias=b2col[:], scale=float(2.0 ** (-S_SCALE)),
                    )
                    for c in range(TB):
                        t = b * TB + c
                        eg = epool.tile([128, 128], bf16, tag="eoh")
                        nc.vector.tensor_scalar(
                            eg[:], iota_row[:], bt[:, t : t + 1],
                            g[:, c : c + 1],
                            op0=mybir.AluOpType.is_equal,
                            op1=mybir.AluOpType.mult,
                        )
                        nc.tensor.matmul(
                            pooled[:], eg[:], xat(t),
                            start=(t == 0), stop=(t == NT - 1),
                            skip_group_check=True,
                        )

                for b in range(NB + LAG):
                    if b < NB:
                        emit_h1(b)
                        emit_fused(b)
                    bl = b - LAG
                    if bl >= 0:
                        emit_gate_pool(bl)

                ps = gsb.tile([GBLK, H], fp32, tag="poolsb")
                nc.vector.tensor_scalar(
                    ps[:], pooled[:], invc[j][:], None,
                    op0=mybir.AluOpType.mult
                )
                pf = fm_copy(ps[:], gsb, "poolfm", bf16)

                # ---- GRU cell (graph-major) ----
                gf, h_old = g_fm, g_gm

                def gru_mm(psum, wi, wh, bias_row, bcol0, bn):
                    mms = []
                    if wi is not None:
                        mms += [(pf[:, ki, :], wi[ki][:, bcol0 : bcol0 + bn])
                                for ki in range(2)]
                    if wh is not None:
                        mms += [(gf[:, ki, :], wh[ki][:, bcol0 : bcol0 + bn])
                                for ki in range(2)]
                    for i, (lhsT, rhs) in enumerate(mms):
                        nc.tensor.matmul(
                            psum[:], lhsT, rhs, start=(i == 0), stop=False,
                            skip_group_check=True,
                        )
                    nc.tensor.matmul(
                        psum[:], ones_bf[:], bias_row, start=False, stop=True,
                        skip_group_check=True,
                    )

                rp = ps_h1.tile([GBLK, H], fp32, tag="psh1")
                gru_mm(rp, wih, whh, brz[:, 0:H], 0, H)
                r = smallsb.tile([GBLK, H], fp32, tag="gru_r")
                nc.scalar.activation(r[:], rp[:],
                                     mybir.ActivationFunctionType.Sigmoid)
                zp = ps_h1.tile([GBLK, H], fp32, tag="psh1")
                gru_mm(zp, wih, whh, brz[:, H : 2 * H], H, H)
                z = smallsb.tile([GBLK, H], fp32, tag="gru_z")
                nc.scalar.activation(z[:], zp[:],
                                     mybir.ActivationFunctionType.Sigmoid)
                inp_ = ps_h1.tile([GBLK, H], fp32, tag="psh1")
                gru_mm(inp_, wih, None, bin_[:], 2 * H, H)
                hnp = ps_h1.tile([GBLK, H], fp32, tag="psh1")
                gru_mm(hnp, None, whh, bhn[:], 2 * H, H)
                t1 = smallsb.tile([GBLK, H], fp32, tag="gru_s1")
                nc.vector.tensor_mul(t1[:], r[:], hnp[:])
                t2 = smallsb.tile([GBLK, H], fp32, tag="gru_s2")
                nc.vector.tensor_add(t2[:], t1[:], inp_[:])
                n = smallsb.tile([GBLK, H], fp32, tag="gru_n")
                nc.scalar.activation(n[:], t2[:],
                                     mybir.ActivationFunctionType.Tanh)
                t3 = smallsb.tile([GBLK, H], fp32, tag="gru_s1")
                nc.vector.tensor_sub(t3[:], h_old[:], n[:])
                t4 = smallsb.tile([GBLK, H], fp32, tag="gru_s2")
                nc.vector.tensor_mul(t4[:], z[:], t3[:])
                t5 = smallsb.tile([GBLK, H], fp32, tag="gru_s3")
                nc.vector.tensor_add(t5[:], n[:], t4[:])
                g_gm = gsb.tile([GBLK, H], fp32, tag="gsb")
                nc.scalar.activation(g_gm[:], t5[:],
                                     mybir.ActivationFunctionType.Relu)
                if ts < NUM_TIMESTEPS - 1:
                    g_fm = fm_copy(g_gm[:], gsb, "gfm", bf16)

            nc.sync.dma_start(out_d[j * GBLK : (j + 1) * GBLK, :], g_gm[:])

    nc.compile()
    return nc


def _prep_inputs(x, batch, counts, n_cores, nblk, NT=None):
    """Host-side shard + pad + layout. Returns (per_core, NT)."""
    import ml_dtypes

    G = n_cores * nblk * GBLK
    batch = np.asarray(batch).astype(np.int64)
    x = np.asarray(x, dtype=np.float32)

    edges = np.searchsorted(batch, np.arange(0, G + 1, GBLK))
    blk_cnt = np.diff(edges)
    if NT is None:
        NT = int(np.ceil(blk_cnt.max() / 128))
        NT = ((NT + TB - 1) // TB) * TB
        if (NT // TB) % 2:
            NT += TB  # halves must hold whole TB batches
    NTP = NT * 128

    invc_all = (1.0 / np.maximum(counts, 1.0)).astype(np.float32)

    xb = x.astype(ml_dtypes.bfloat16)
    x8 = x.astype(ml_dtypes.float8_e4m3)
    per_core = []
    for k in range(n_cores):
        xk = np.zeros((nblk * NTP, H), dtype=ml_dtypes.bfloat16)
        xk8 = np.zeros((nblk, NTP, H), dtype=ml_dtypes.float8_e4m3)
        bcols = np.full((nblk, 128, NT), -1.0, dtype=np.float32)
        for j in range(nblk):
            bi = k * nblk + j
            lo, hi = edges[bi], edges[bi + 1]
            cnt = hi - lo
            xk[j * NTP : j * NTP + cnt] = xb[lo:hi]
            xk8[j, :cnt] = x8[lo:hi]
            blp = np.full(NTP, -1.0, dtype=np.float32)
            blp[:cnt] = (batch[lo:hi] - (bi * GBLK)).astype(np.float32)
            bcols[j] = blp.reshape(NT, 128).T
        # [j, p(feat in ktile), t, k(ktile), n] = x8[j, t*128+n, k*128+p]
        xkT8 = np.ascontiguousarray(
            xk8.reshape(nblk, NT, 128, 2, 128).transpose(0, 4, 1, 3, 2)
        )
        invc = invc_all[k * nblk * GBLK : (k + 1) * nblk * GBLK].reshape(
            nblk, GBLK, 1
        )
        per_core.append({"xk": xk, "xkT8": xkT8, "bcols": bcols,
                         "invc": np.ascontiguousarray(invc)})
    return per_core, NT


def _const_inputs(gate_w1, gate_b1, gate_w2, gate_b2, gru_w_ih, gru_w_hh,
                  gru_b_ih, gru_b_hh):
    import ml_dtypes

    f = np.float32
    bf = ml_dtypes.bfloat16
    f8 = ml_dtypes.float8_e4m3
    c = {}
    w1 = np.asarray(gate_w1, f)  # (H, H), h1 = x @ w1.T
    w2 = np.asarray(gate_w2, f).reshape(H)
    # permute h1 columns so positive-w2 ones come first (sign-split drains)
    perm = np.concatenate([np.where(w2 >= 0.0)[0], np.where(w2 < 0.0)[0]])
    npos = int((w2 >= 0.0).sum())
    w1 = w1[perm]
    w2 = w2[perm]
    scale = (2.0 ** S_SCALE)
    w1p = w1 * (np.abs(w2)[:, None] * scale)  # fold |w2| into rows of W1
    b1p = np.asarray(gate_b1, f)[perm] * np.abs(w2) * scale
    sgn = np.where(w2 >= 0.0, 1.0, -1.0).astype(f)
    c["_npos"] = npos
    # w1dr[p, k, h] = w1p[h, k*128+p]
    c["w1dr"] = np.ascontiguousarray(
        w1p.T.reshape(2, 128, H).transpose(1, 0, 2)).astype(f8)
    c["w1tp"] = np.ascontiguousarray(w1p.T.reshape(2, 128, H)).astype(bf)
    c["b1prow"] = b1p.reshape(1, H).astype(bf)
    c["sgnbc"] = np.tile(sgn.reshape(1, H), (128, 1)).astype(bf)
    c["b2col"] = np.full((128, 1), np.asarray(gate_b2, f).reshape(()), dtype=f)
    c["wih_t"] = np.ascontiguousarray(
        np.asarray(gru_w_ih, f).T).reshape(2, 128, 3 * H).astype(bf)
    c["whh_t"] = np.ascontiguousarray(
        np.asarray(gru_w_hh, f).T).reshape(2, 128, 3 * H).astype(bf)
    bih = np.asarray(gru_b_ih, f)
    bhh = np.asarray(gru_b_hh, f)
    c["bsum_rz"] = (bih[: 2 * H] + bhh[: 2 * H]).reshape(1, 2 * H).astype(bf)
    c["bihn"] = bih[2 * H :].reshape(1, H).astype(bf)
    c["bhhn"] = bhh[2 * H :].reshape(1, H).astype(bf)
    c["iota_row"] = np.tile(np.arange(128, dtype=f), (128, 1)).astype(bf)
    c["eye128"] = np.eye(128, dtype=f)
    c["eye128b"] = np.eye(128, dtype=f).astype(bf)
    return c


_CACHE = {}


def run(x, gate_w1, gate_b1, gate_w2, gate_b2, gru_w_ih, gru_w_hh, gru_b_ih,
        gru_b_hh, batch, num_graphs, n_cores=8, nblk=NBLK, trace=False,
        use_sim=False):
    from concourse.bass_utils import run_bass_kernel_spmd

    batch = np.asarray(batch).astype(np.int64)
    G = n_cores * nblk * GBLK
    counts = np.bincount(batch, minlength=G).astype(np.float32)
    per_core, NT = _prep_inputs(x, batch, counts, n_cores, nblk)
    consts = _const_inputs(gate_w1, gate_b1, gate_w2, gate_b2, gru_w_ih,
                           gru_w_hh, gru_b_ih, gru_b_hh)
    npos = consts.pop("_npos")
    in_maps = [{**consts, **pc} for pc in per_core]

    key = (NT, npos, nblk, n_cores)
    if key not in _CACHE:
        _CACHE[key] = _build_program(NT, npos, nblk=nblk)
    nc = _CACHE[key]

    if use_sim:
        from concourse.bass_interp import CoreSim

        outs = []
        for k in range(n_cores):
            sim = CoreSim(nc)
            for name, arr in in_maps[k].items():
                sim.tensor(name)[:] = arr
            sim.simulate()
            outs.append(np.array(sim.tensor("out")))
        return np.concatenate(outs, axis=0), None

    res = run_bass_kernel_spmd(nc, in_maps, core_ids=list(range(n_cores)),
                               trace=trace)
    out = np.concatenate([res.results[k]["out"] for k in range(n_cores)], axis=0)
    return out, res


def kernel(**inputs):
    out, _ = run(**inputs)
    return out


# revision 37
# speedup vs baseline: 2.3511x; 1.1402x over previous
"""Trainium2 Bass kernel for AttentiveGraphPooling (gnn_message_passing).

Strategy: shard the 4096 graphs across 8 cores (512 graphs each). batch is
sorted, so each core owns a contiguous node range covering whole graphs ->
pooling / gather / GRU are all core-local, no collectives needed.

Per core, graphs go in 4 blocks of 128; nodes in NT tiles of 128 per block.
Resident per block: x node-major (bf16, for pooling), x feature-major in
fp8 DoubleRow layout (for the gate matmul), E^T one-hot (fp8, built once in
phase A via PE transpose).

Gate math: |w2| (and a 2^s scale) is folded into W1's columns host-side, so
  logit = sum_h sign(w2)_h * relu(h1'[n,h]),  h1' = x@W1'^T + GW1'[b(n)]
h1' is computed per node tile as ONE fp8 DoubleRow matmul (K=256) plus one
DoubleRow gather matmul (E^T tile paired with a zero k-tile). The whole
relu->*w2->reduce chain is ONE fused scalar_tensor_tensor (op0=max 0,
op1=mult sign-row, accum_out=logit column), split between the Vector and
GpSimd engines. Gates come from one batched sigmoid (scale=2^-s, bias=b2).

Weighted pooling stays bf16 (fp8 would lose too much precision in the
mean's cancellation): Eg one-hot built on DVE fused is_eq*gate, one matmul
per tile. Pool matmuls are emitted with a 2-batch lag so the PE runs long
uninterrupted bursts (p-state stays high).
"""

import os
import sys

import numpy as np

sys.path.insert(0, "/opt/trn_rl_repo")

H = 256
NBLK = 4  # graph blocks per core
GBLK = 128  # graphs per block
NUM_TIMESTEPS = 2
LCHUNK = 16  # node tiles per resident-load DMA
TB = 8  # gate batch (node tiles per sigmoid batch)
LAG = 2  # batches of lag for pool-matmul emission (PE burst length)
S_SCALE = 7  # 2^s fold into W1' so fp8 entries are in range

# fraction of gate batches drained by the Scalar engine (sign-split relu
# accumulate); the rest use the DVE fused op
ACT_NUM = 0
ACT_DEN = 5


def _build_program(NT, npos, nblk=NBLK):
    """Build the single-core SPMD Bass program. NT = node tiles per block."""
    from contextlib import ExitStack

    import concourse.bass as bass
    import concourse.tile as tile
    from concourse import bacc, mybir

    fp32 = mybir.dt.float32
    bf16 = mybir.dt.bfloat16
    fp8 = mybir.dt.float8e4
    DR = mybir.MatmulPerfMode.DoubleRow

    NTP = NT * 128  # padded nodes per block
    NB = NT // TB  # gate batches per block

    nc = bacc.Bacc("TRN2", target_bir_lowering=False, debug=False)

    # ---- DRAM parameters (per-core inputs) ----
    assert 0 < npos < H
    x_d = nc.dram_tensor("xk", [nblk * NTP, H], bf16, kind="ExternalInput")
    xt_d = nc.dram_tensor("xkT8", [nblk, 128, NT, 2, 128], fp8, kind="ExternalInput")
    bcols_d = nc.dram_tensor("bcols", [nblk, 128, NT], fp32, kind="ExternalInput")
    invc_d = nc.dram_tensor("invc", [nblk, GBLK, 1], fp32, kind="ExternalInput")
    w1dr_d = nc.dram_tensor("w1dr", [128, 2, H], fp8, kind="ExternalInput")
    w1tp_d = nc.dram_tensor("w1tp", [2, 128, H], bf16, kind="ExternalInput")
    b1p_d = nc.dram_tensor("b1prow", [1, H], bf16, kind="ExternalInput")
    sgn_d = nc.dram_tensor("sgnbc", [128, H], bf16, kind="ExternalInput")
    b2c_d = nc.dram_tensor("b2col", [128, 1], fp32, kind="ExternalInput")
    wih_d = nc.dram_tensor("wih_t", [2, 128, 3 * H], bf16, kind="ExternalInput")
    whh_d = nc.dram_tensor("whh_t", [2, 128, 3 * H], bf16, kind="ExternalInput")
    brz_d = nc.dram_tensor("bsum_rz", [1, 2 * H], bf16, kind="ExternalInput")
    bin_d = nc.dram_tensor("bihn", [1, H], bf16, kind="ExternalInput")
    bhn_d = nc.dram_tensor("bhhn", [1, H], bf16, kind="ExternalInput")
    iota_d = nc.dram_tensor("iota_row", [128, 128], bf16, kind="ExternalInput")
    eye_d = nc.dram_tensor("eye128", [128, 128], fp32, kind="ExternalInput")
    eyeb_d = nc.dram_tensor("eye128b", [128, 128], bf16, kind="ExternalInput")
    out_d = nc.dram_tensor("out", [nblk * GBLK, H], fp32, kind="ExternalOutput")

    with tile.TileContext(nc) as tc, ExitStack() as ctx:
        ep = ctx.enter_context  # shorthand

        const = ep(tc.tile_pool(name="const", bufs=1))
        xres = ep(tc.tile_pool(name="xres", bufs=3))
        xtres = ep(tc.tile_pool(name="xtres", bufs=2))
        etres = ep(tc.tile_pool(name="etres", bufs=1))
        bpool = ep(tc.tile_pool(name="bcols", bufs=2))
        epool = ep(tc.tile_pool(name="eoh", bufs=6))
        scr = ep(tc.tile_pool(name="scr", bufs=4))
        gtsb = ep(tc.tile_pool(name="gtsb", bufs=4))
        gsb = ep(tc.tile_pool(name="gsb", bufs=2))
        smallsb = ep(tc.tile_pool(name="smallsb", bufs=1))

        ps_pool = ep(tc.tile_pool(name="pspool", bufs=2, space="PSUM"))
        ps_h1 = ep(tc.tile_pool(name="psh1", bufs=4, space="PSUM"))
        ps_sm = ep(tc.tile_pool(name="pssm", bufs=2, space="PSUM"))

        # ---- load constants ----
        def cload(shape, src, tag, dt=fp32):
            t = const.tile(shape, dt, tag=tag)
            nc.sync.dma_start(t[:], src)
            return t

        iota_row = cload([128, 128], iota_d[:], "c_iota", bf16)
        eye = cload([128, 128], eye_d[:], "c_eye")
        eyeb = cload([128, 128], eyeb_d[:], "c_eyeb", bf16)
        w1dr = cload([128, 2, H], w1dr_d[:], "c_w1dr", fp8)
        w1tp = [cload([128, H], w1tp_d[k], f"c_w1tp{k}", bf16) for k in range(2)]
        b1p = cload([1, H], b1p_d[:], "c_b1p", bf16)
        sgnbc = cload([128, H], sgn_d[:], "c_sgn", bf16)
        b2col = cload([128, 1], b2c_d[:], "c_b2c")
        wih = [cload([128, 3 * H], wih_d[k], f"c_wih{k}", bf16) for k in range(2)]
        whh = [cload([128, 3 * H], whh_d[k], f"c_whh{k}", bf16) for k in range(2)]
        brz = cload([1, 2 * H], brz_d[:], "c_brz", bf16)
        bin_ = cload([1, H], bin_d[:], "c_bin", bf16)
        bhn = cload([1, H], bhn_d[:], "c_bhn", bf16)
        invc = [cload([GBLK, 1], invc_d[j], f"c_invc{j}") for j in range(nblk)]
        ones_row = const.tile([1, 128], fp32)
        nc.vector.memset(ones_row[:], 1.0)
        ones_bf = const.tile([1, 128], bf16)
        nc.vector.memset(ones_bf[:], 1.0)

        def fm_copy(g_ap, pool, tag, dt):
            """(128,256) graph-major -> feature-major (128,2,128) via PE."""
            gf = pool.tile([128, 2, GBLK], dt, tag=tag)
            tp = ps_sm.tile([128, 2, 128], fp32, tag="pssm")
            for ki in range(2):
                nc.tensor.matmul(tp[:, ki, :], g_ap[:, ki * 128 : (ki + 1) * 128],
                                 eye[:], is_transpose=True, start=True, stop=True)
                nc.scalar.copy(gf[:, ki, :], tp[:, ki, :])
            return gf

        NH = NT // 2  # resident x split in halves so the next block's first
        # half can load while this block's second half is still in use

        def fused_logit(h1p_ap, scratch_ap, acc_ap):
            """accum(relu(h1) * sign-row); DVE only (GPSIMD can't read PSUM)."""
            nc.vector.scalar_tensor_tensor(
                scratch_ap, h1p_ap, 0.0, sgnbc[:],
                op0=mybir.AluOpType.max, op1=mybir.AluOpType.mult,
                accum_out=acc_ap,
            )

        for j in range(nblk):
            # bcols first: it unblocks all one-hot builds
            bt = bpool.tile([128, NT], fp32, tag="bcols")
            nc.sync.dma_start(bt[:], bcols_d[j])

            # ---- resident x (node-major bf16) for this block, two halves ----
            xhalves = []
            for h0 in (0, NH):
                xh = xres.tile([128, NH, H], bf16, tag="xres")
                for c0 in range(0, NH, LCHUNK):
                    cn = min(LCHUNK, NH - c0)
                    base = j * NTP + (h0 + c0) * 128
                    src = x_d[base : base + cn * 128, :].rearrange(
                        "(c p) h -> p c h", p=128
                    )
                    nc.sync.dma_start(xh[:, c0 : c0 + cn, :], src)
                xhalves.append(xh)

            def xat(t):
                return xhalves[t // NH][:, t % NH, :]

            # ---- resident x^T fp8 DoubleRow layout, two halves ----
            xthalves = []
            for h0 in (0, NH):
                xth = xtres.tile([128, NH, 2, 128], fp8, tag="xtres")
                for c0 in range(0, NH, LCHUNK):
                    cn = min(LCHUNK, NH - c0)
                    nc.sync.dma_start(
                        xth[:, c0 : c0 + cn, :, :],
                        xt_d[j, :, h0 + c0 : h0 + c0 + cn, :, :],
                    )
                xthalves.append(xth)

            def xtat(t):
                return xthalves[t // NH][:, t % NH, :, :]

            etj = etres.tile([128, NT, 128], bf16, tag="etres")

            # ---- phase A: initial mean pool + resident E^T build ----
            # e-builds run ahead (DVE); E^T via XBAR DMA transpose; pool
            # matmuls lag so the PE sees contiguous work.
            pooled = ps_pool.tile([GBLK, H], fp32, tag="pspool")
            ephA = []
            for t in range(NT + LAG):
                if t < NT:
                    e = epool.tile([128, 128], bf16, tag="eoh")
                    nc.vector.tensor_scalar(
                        e[:], iota_row[:], bt[:, t : t + 1], None,
                        op0=mybir.AluOpType.is_equal,
                    )
                    nc.sync.dma_start_transpose(out=etj[:, t, :], in_=e[:])
                    ephA.append(e)
                tl = t - LAG
                if tl >= 0:
                    e = ephA[tl]
                    nc.tensor.matmul(
                        pooled[:], e[:], xat(tl), start=(tl == 0),
                        stop=(tl == NT - 1), skip_group_check=True,
                    )
            g_gm = gsb.tile([GBLK, H], fp32, tag="gsb")
            nc.vector.tensor_scalar(
                g_gm[:], pooled[:], invc[j][:], None, op0=mybir.AluOpType.mult
            )
            g_fm = fm_copy(g_gm[:], gsb, "gfm", bf16)

            # ---- timesteps ----
            for ts in range(NUM_TIMESTEPS):
                # GW1' = G @ W1'^T + b1'  (graph-level, bf16 -> fp8 copy)
                gw1p = ps_sm.tile([GBLK, H], fp32, tag="pssm")
                for ki in range(2):
                    nc.tensor.matmul(gw1p[:], g_fm[:, ki, :], w1tp[ki][:],
                                     start=(ki == 0), stop=False,
                                     skip_group_check=True)
                nc.tensor.matmul(gw1p[:], ones_bf[:], b1p[:],
                                 start=False, stop=True, skip_group_check=True)
                gw1bf = gsb.tile([GBLK, H], bf16, tag="gw1bf")
                nc.scalar.copy(gw1bf[:], gw1p[:])

                pooled = ps_pool.tile([GBLK, H], fp32, tag="pspool")

                # software-pipelined batches: h1 mms + fused logit run
                # ahead; sigmoid, eg builds and pool matmuls lag.
                h1ps = {}
                gacc = {}
                gt = {}

                def is_act_batch(b):
                    return (b % ACT_DEN) < ACT_NUM

                def emit_h1(b):
                    gaP = gtsb.tile([128, TB], fp32, tag="gacc", name="gaP")
                    gaN = (gtsb.tile([128, TB], fp32, tag="gaccN", name="gaN")
                           if is_act_batch(b) else None)
                    gacc[b] = (gaP, gaN)
                    for c in range(TB):
                        t = b * TB + c
                        if c % 2 == 0:
                            h1pair = ps_h1.tile([128, 2, H], fp32, tag="psh1")
                        h1ps[t] = h1pair[:, c % 2, :]
                        nc.tensor.matmul(h1ps[t], xtat(t), w1dr[:],
                                         start=True, stop=False, perf_mode=DR)
                        nc.tensor.matmul(h1ps[t], etj[:, t, :], gw1bf[:],
                                         start=False, stop=True)

                def emit_fused(b):
                    gaP, gaN = gacc[b]
                    for c in range(TB):
                        t = b * TB + c
                        if gaN is not None:
                            # sign-split relu accumulate on the Scalar engine
                            scp = scr.tile([128, npos], bf16, tag="scrp")
                            nc.scalar.activation(
                                scp[:], h1ps[t][:, :npos],
                                mybir.ActivationFunctionType.Relu,
                                accum_out=gaP[:, c : c + 1],
                            )
                            scn = scr.tile([128, H - npos], bf16, tag="scrn")
                            nc.scalar.activation(
                                scn[:], h1ps[t][:, npos:],
                                mybir.ActivationFunctionType.Relu,
                                accum_out=gaN[:, c : c + 1],
                            )
                        else:
                            sc = scr.tile([128, H], bf16, tag="scr")
                            fused_logit(h1ps[t], sc[:], gaP[:, c : c + 1])
                        del h1ps[t]

                def emit_gate_pool(b):
                    gaP, gaN = gacc[b]
                    if gaN is not None:
                        gd = gtsb.tile([128, TB], fp32, tag="gacc")
                        nc.vector.tensor_sub(gd[:], gaP[:], gaN[:])
                        gaP = gd
                    g = gtsb.tile([128, TB], fp32, tag="gt")
                    gt[b] = g
                    nc.scalar.activation(
                        g[:], gaP[:],
                        mybir.ActivationFunctionType.Sigmoid,
                        bias=b2col[:], scale=float(2.0 ** (-S_SCALE)),
                    )
                    for c in range(TB):
                        t = b * TB + c
                        eg = epool.tile([128, 128], bf16, tag="eoh")
                        nc.vector.tensor_scalar(
                            eg[:], iota_row[:], bt[:, t : t + 1],
                            g[:, c : c + 1],
                            op0=mybir.AluOpType.is_equal,
                            op1=mybir.AluOpType.mult,
                        )
                        nc.tensor.matmul(
                            pooled[:], eg[:], xat(t),
                            start=(t == 0), stop=(t == NT - 1),
                            skip_group_check=True,
                        )

                for b in range(NB + LAG):
                    if b < NB:
                        emit_h1(b)
                        emit_fused(b)
                    bl = b - LAG
                    if bl >= 0:
                        emit_gate_pool(bl)

                ps = gsb.tile([GBLK, H], fp32, tag="poolsb")
                nc.vector.tensor_scalar(
                    ps[:], pooled[:], invc[j][:], None,
                    op0=mybir.AluOpType.mult
                )
                pf = fm_copy(ps[:], gsb, "poolfm", bf16)

                # ---- GRU cell (graph-major) ----
                gf, h_old = g_fm, g_gm

                def gru_mm(psum, wi, wh, bias_row, bcol0, bn):
                    mms = []
                    if wi is not None:
                        mms += [(pf[:, ki, :], wi[ki][:, bcol0 : bcol0 + bn])
                                for ki in range(2)]
                    if wh is not None:
                        mms += [(gf[:, ki, :], wh[ki][:, bcol0 : bcol0 + bn])
                                for ki in range(2)]
                    for i, (lhsT, rhs) in enumerate(mms):
                        nc.tensor.matmul(
                            psum[:], lhsT, rhs, start=(i == 0), stop=False,
                            skip_group_check=True,
                        )
                    nc.tensor.matmul(
                        psum[:], ones_bf[:], bias_row, start=False, stop=True,
                        skip_group_check=True,
                    )

                rp = ps_h1.tile([GBLK, H], fp32, tag="psh1")
                gru_mm(rp, wih, whh, brz[:, 0:H], 0, H)
                r = smallsb.tile([GBLK, H], fp32, tag="gru_r")
                nc.scalar.activation(r[:], rp[:],
                                     mybir.ActivationFunctionType.Sigmoid)
                zp = ps_h1.tile([GBLK, H], fp32, tag="psh1")
                gru_mm(zp, wih, whh, brz[:, H : 2 * H], H, H)
                z = smallsb.tile([GBLK, H], fp32, tag="gru_z")
                nc.scalar.activation(z[:], zp[:],
                                     mybir.ActivationFunctionType.Sigmoid)
                inp_ = ps_h1.tile([GBLK, H], fp32, tag="psh1")
                gru_mm(inp_, wih, None, bin_[:], 2 * H, H)
                hnp = ps_h1.tile([GBLK, H], fp32, tag="psh1")
                gru_mm(hnp, None, whh, bhn[:], 2 * H, H)
                t1 = smallsb.tile([GBLK, H], fp32, tag="gru_s1")
                nc.vector.tensor_mul(t1[:], r[:], hnp[:])
                t2 = smallsb.tile([GBLK, H], fp32, tag="gru_s2")
                nc.vector.tensor_add(t2[:], t1[:], inp_[:])
                n = smallsb.tile([GBLK, H], fp32, tag="gru_n")
                nc.scalar.activation(n[:], t2[:],
                                     mybir.ActivationFunctionType.Tanh)
                t3 = smallsb.tile([GBLK, H], fp32, tag="gru_s1")
                nc.vector.tensor_sub(t3[:], h_old[:], n[:])
                t4 = smallsb.tile([GBLK, H], fp32, tag="gru_s2")
                nc.vector.tensor_mul(t4[:], z[:], t3[:])
                t5 = smallsb.tile([GBLK, H], fp32, tag="gru_s3")
                nc.vector.tensor_add(t5[:], n[:], t4[:])
                g_gm = gsb.tile([GBLK, H], fp32, tag="gsb")
                nc.scalar.activation(g_gm[:], t5[:],
                                     mybir.ActivationFunctionType.Relu)
                if ts < NUM_TIMESTEPS - 1:
                    g_fm = fm_copy(g_gm[:], gsb, "gfm", bf16)

            nc.sync.dma_start(out_d[j * GBLK : (j + 1) * GBLK, :], g_gm[:])

    nc.compile()
    return nc


def _prep_inputs(x, batch, counts, n_cores, nblk, NT=None):
    """Host-side shard + pad + layout. Returns (per_core, NT)."""
    import ml_dtypes

    G = n_cores * nblk * GBLK
    batch = np.asarray(batch).astype(np.int64)
    x = np.asarray(x, dtype=np.float32)

    edges = np.searchsorted(batch, np.arange(0, G + 1, GBLK))
    blk_cnt = np.diff(edges)
    if NT is None:
        NT = int(np.ceil(blk_cnt.max() / 128))
        NT = ((NT + TB - 1) // TB) * TB
        if (NT // TB) % 2:
            NT += TB  # halves must hold whole TB batches
    NTP = NT * 128

    invc_all = (1.0 / np.maximum(counts, 1.0)).astype(np.float32)

    xb = x.astype(ml_dtypes.bfloat16)
    x8 = x.astype(ml_dtypes.float8_e4m3)
    per_core = []
    for k in range(n_cores):
        xk = np.zeros((nblk * NTP, H), dtype=ml_dtypes.bfloat16)
        xk8 = np.zeros((nblk, NTP, H), dtype=ml_dtypes.float8_e4m3)
        bcols = np.full((nblk, 128, NT), -1.0, dtype=np.float32)
        for j in range(nblk):
            bi = k * nblk + j
            lo, hi = edges[bi], edges[bi + 1]
            cnt = hi - lo
            xk[j * NTP : j * NTP + cnt] = xb[lo:hi]
            xk8[j, :cnt] = x8[lo:hi]
            blp = np.full(NTP, -1.0, dtype=np.float32)
            blp[:cnt] = (batch[lo:hi] - (bi * GBLK)).astype(np.float32)
            bcols[j] = blp.reshape(NT, 128).T
        # [j, p(feat in ktile), t, k(ktile), n] = x8[j, t*128+n, k*128+p]
        xkT8 = np.ascontiguousarray(
            xk8.reshape(nblk, NT, 128, 2, 128).transpose(0, 4, 1, 3, 2)
        )
        invc = invc_all[k * nblk * GBLK : (k + 1) * nblk * GBLK].reshape(
            nblk, GBLK, 1
        )
        per_core.append({"xk": xk, "xkT8": xkT8, "bcols": bcols,
                         "invc": np.ascontiguousarray(invc)})
    return per_core, NT


def _const_inputs(gate_w1, gate_b1, gate_w2, gate_b2, gru_w_ih, gru_w_hh,
                  gru_b_ih, gru_b_hh):
    import ml_dtypes

    f = np.float32
    bf = ml_dtypes.bfloat16
    f8 = ml_dtypes.float8_e4m3
    c = {}
    w1 = np.asarray(gate_w1, f)  # (H, H), h1 = x @ w1.T
    w2 = np.asarray(gate_w2, f).reshape(H)
    # permute h1 columns so positive-w2 ones come first (sign-split drains)
    perm = np.concatenate([np.where(w2 >= 0.0)[0], np.where(w2 < 0.0)[0]])
    npos = int((w2 >= 0.0).sum())
    w1 = w1[perm]
    w2 = w2[perm]
    scale = (2.0 ** S_SCALE)
    w1p = w1 * (np.abs(w2)[:, None] * scale)  # fold |w2| into rows of W1
    b1p = np.asarray(gate_b1, f)[perm] * np.abs(w2) * scale
    sgn = np.where(w2 >= 0.0, 1.0, -1.0).astype(f)
    c["_npos"] = npos
    # w1dr[p, k, h] = w1p[h, k*128+p]
    c["w1dr"] = np.ascontiguousarray(
        w1p.T.reshape(2, 128, H).transpose(1, 0, 2)).astype(f8)
    c["w1tp"] = np.ascontiguousarray(w1p.T.reshape(2, 128, H)).astype(bf)
    c["b1prow"] = b1p.reshape(1, H).astype(bf)
    c["sgnbc"] = np.tile(sgn.reshape(1, H), (128, 1)).astype(bf)
    c["b2col"] = np.full((128, 1), np.asarray(gate_b2, f).reshape(()), dtype=f)
    c["wih_t"] = np.ascontiguousarray(
        np.asarray(gru_w_ih, f).T).reshape(2, 128, 3 * H).astype(bf)
    c["whh_t"] = np.ascontiguousarray(
        np.asarray(gru_w_hh, f).T).reshape(2, 128, 3 * H).astype(bf)
    bih = np.asarray(gru_b_ih, f)
    bhh = np.asarray(gru_b_hh, f)
    c["bsum_rz"] = (bih[: 2 * H] + bhh[: 2 * H]).reshape(1, 2 * H).astype(bf)
    c["bihn"] = bih[2 * H :].reshape(1, H).astype(bf)
    c["bhhn"] = bhh[2 * H :].reshape(1, H).astype(bf)
    c["iota_row"] = np.tile(np.arange(128, dtype=f), (128, 1)).astype(bf)
    c["eye128"] = np.eye(128, dtype=f)
    c["eye128b"] = np.eye(128, dtype=f).astype(bf)
    return c


_CACHE = {}


def run(x, gate_w1, gate_b1, gate_w2, gate_b2, gru_w_ih, gru_w_hh, gru_b_ih,
        gru_b_hh, batch, num_graphs, n_cores=8, nblk=NBLK, trace=False,
        use_sim=False):
    from concourse.bass_utils import run_bass_kernel_spmd

    batch = np.asarray(batch).astype(np.int64)
    G = n_cores * nblk * GBLK
    counts = np.bincount(batch, minlength=G).astype(np.float32)
    per_core, NT = _prep_inputs(x, batch, counts, n_cores, nblk)
    consts = _const_inputs(gate_w1, gate_b1, gate_w2, gate_b2, gru_w_ih,
                           gru_w_hh, gru_b_ih, gru_b_hh)
    npos = consts.pop("_npos")
    in_maps = [{**consts, **pc} for pc in per_core]

    key = (NT, npos, nblk, n_cores)
    if key not in _CACHE:
        _CACHE[key] = _build_program(NT, npos, nblk=nblk)
    nc = _CACHE[key]

    if use_sim:
        from concourse.bass_interp import CoreSim

        outs = []
        for k in range(n_cores):
            sim = CoreSim(nc)
            for name, arr in in_maps[k].items():
                sim.tensor(name)[:] = arr
            sim.simulate()
            outs.append(np.array(sim.tensor("out")))
        return np.concatenate(outs, axis=0), None

    res = run_bass_kernel_spmd(nc, in_maps, core_ids=list(range(n_cores)),
                               trace=trace)
    out = np.concatenate([res.results[k]["out"] for k in range(n_cores)], axis=0)
    return out, res


def kernel(**inputs):
    out, _ = run(**inputs)
    return out


# revision 47
# speedup vs baseline: 3.9656x; 1.6867x over previous
"""Trainium2 Bass kernel for AttentiveGraphPooling (gnn_message_passing).

Strategy: shard the 4096 graphs across 8 cores (512 graphs each). batch is
sorted, so each core owns a contiguous node range covering whole graphs ->
pooling / gather / GRU are all core-local, no collectives needed.

Per core, graphs go in 4 blocks of 128; nodes in NT tiles of 128 per block.
Resident per block: x node-major (bf16, for pooling), x feature-major in
fp8 DoubleRow layout (for the gate matmul), E^T one-hot (fp8, built once in
phase A via PE transpose).

Gate math: |w2| (and a 2^s scale) is folded into W1's columns host-side, so
  logit = sum_h sign(w2)_h * relu(h1'[n,h]),  h1' = x@W1'^T + GW1'[b(n)]
h1' is computed per node tile as ONE fp8 DoubleRow matmul (K=256) plus one
DoubleRow gather matmul (E^T tile paired with a zero k-tile). The whole
relu->*w2->reduce chain is ONE fused scalar_tensor_tensor (op0=max 0,
op1=mult sign-row, accum_out=logit column), split between the Vector and
GpSimd engines. Gates come from one batched sigmoid (scale=2^-s, bias=b2).

Weighted pooling stays bf16 (fp8 would lose too much precision in the
mean's cancellation): Eg one-hot built on DVE fused is_eq*gate, one matmul
per tile. Pool matmuls are emitted with a 2-batch lag so the PE runs long
uninterrupted bursts (p-state stays high).
"""

import os
import sys

import numpy as np

sys.path.insert(0, "/opt/trn_rl_repo")

H = 256
NBLK = 4  # graph blocks per core
GBLK = 128  # graphs per block
NUM_TIMESTEPS = 2
LCHUNK = 16  # node tiles per resident-load DMA
TB = 8  # gate batch (node tiles per sigmoid batch)
LAG = 2  # batches of lag for pool-matmul emission (PE burst length)
S_SCALE = 7  # 2^s fold into W1' so fp8 entries are in range

# fraction of gate batches drained via act-relu + DVE 2x sign-dot;
# the rest use the DVE psum-direct fused op
ACT_NUM = 5
ACT_DEN = 6


def _build_program(NT, npos, nblk=NBLK):
    """Build the single-core SPMD Bass program. NT = node tiles per block."""
    from contextlib import ExitStack

    import concourse.bass as bass
    import concourse.tile as tile
    from concourse import bacc, mybir

    fp32 = mybir.dt.float32
    bf16 = mybir.dt.bfloat16
    fp8 = mybir.dt.float8e4
    DR = mybir.MatmulPerfMode.DoubleRow

    NTP = NT * 128  # padded nodes per block
    NB = NT // TB  # gate batches per block

    nc = bacc.Bacc("TRN2", target_bir_lowering=False, debug=False)

    # ---- DRAM parameters (per-core inputs) ----
    assert 0 < npos < H
    x_d = nc.dram_tensor("xk", [nblk * NTP, H], bf16, kind="ExternalInput")
    xt_d = nc.dram_tensor("xkT8", [nblk, 128, NT, 2, 128], fp8, kind="ExternalInput")
    bcols_d = nc.dram_tensor("bcols", [nblk, 128, NT], fp32, kind="ExternalInput")
    invc_d = nc.dram_tensor("invc", [nblk, GBLK, 1], fp32, kind="ExternalInput")
    w1dr_d = nc.dram_tensor("w1dr", [128, 2, H], fp8, kind="ExternalInput")
    w1tp_d = nc.dram_tensor("w1tp", [2, 128, H], bf16, kind="ExternalInput")
    b1p_d = nc.dram_tensor("b1prow", [1, H], bf16, kind="ExternalInput")
    sgn_d = nc.dram_tensor("sgnbc", [128, H], bf16, kind="ExternalInput")
    b2c_d = nc.dram_tensor("b2col", [128, 1], fp32, kind="ExternalInput")
    wih_d = nc.dram_tensor("wih_t", [2, 128, 3 * H], bf16, kind="ExternalInput")
    whh_d = nc.dram_tensor("whh_t", [2, 128, 3 * H], bf16, kind="ExternalInput")
    brz_d = nc.dram_tensor("bsum_rz", [1, 2 * H], bf16, kind="ExternalInput")
    bin_d = nc.dram_tensor("bihn", [1, H], bf16, kind="ExternalInput")
    bhn_d = nc.dram_tensor("bhhn", [1, H], bf16, kind="ExternalInput")
    iota_d = nc.dram_tensor("iota_row", [128, 128], bf16, kind="ExternalInput")
    eye_d = nc.dram_tensor("eye128", [128, 128], fp32, kind="ExternalInput")
    eyeb_d = nc.dram_tensor("eye128b", [128, 128], bf16, kind="ExternalInput")
    out_d = nc.dram_tensor("out", [nblk * GBLK, H], fp32, kind="ExternalOutput")

    with tile.TileContext(nc) as tc, ExitStack() as ctx:
        ep = ctx.enter_context  # shorthand

        const = ep(tc.tile_pool(name="const", bufs=1))
        xres = ep(tc.tile_pool(name="xres", bufs=2))
        xtres = ep(tc.tile_pool(name="xtres", bufs=2))
        etres = ep(tc.tile_pool(name="etres", bufs=2))
        bpool = ep(tc.tile_pool(name="bcols", bufs=2))
        epool = ep(tc.tile_pool(name="eoh", bufs=6))
        scr = ep(tc.tile_pool(name="scr", bufs=6))
        gtsb = ep(tc.tile_pool(name="gtsb", bufs=4))
        gsb = ep(tc.tile_pool(name="gsb", bufs=2))
        smallsb = ep(tc.tile_pool(name="smallsb", bufs=1))

        ps_pool = ep(tc.tile_pool(name="pspool", bufs=2, space="PSUM"))
        ps_h1 = ep(tc.tile_pool(name="psh1", bufs=4, space="PSUM"))
        ps_sm = ep(tc.tile_pool(name="pssm", bufs=2, space="PSUM"))

        # ---- load constants ----
        def cload(shape, src, tag, dt=fp32):
            t = const.tile(shape, dt, tag=tag)
            nc.sync.dma_start(t[:], src)
            return t

        iota_row = cload([128, 128], iota_d[:], "c_iota", bf16)
        eye = cload([128, 128], eye_d[:], "c_eye")
        eyeb = cload([128, 128], eyeb_d[:], "c_eyeb", bf16)
        w1dr = cload([128, 2, H], w1dr_d[:], "c_w1dr", fp8)
        w1tp = [cload([128, H], w1tp_d[k], f"c_w1tp{k}", bf16) for k in range(2)]
        b1p = cload([1, H], b1p_d[:], "c_b1p", bf16)
        sgnbc = cload([128, H], sgn_d[:], "c_sgn", bf16)
        b2col = cload([128, 1], b2c_d[:], "c_b2c")
        wih = [cload([128, 3 * H], wih_d[k], f"c_wih{k}", bf16) for k in range(2)]
        whh = [cload([128, 3 * H], whh_d[k], f"c_whh{k}", bf16) for k in range(2)]
        brz = cload([1, 2 * H], brz_d[:], "c_brz", bf16)
        bin_ = cload([1, H], bin_d[:], "c_bin", bf16)
        bhn = cload([1, H], bhn_d[:], "c_bhn", bf16)
        invc = [cload([GBLK, 1], invc_d[j], f"c_invc{j}") for j in range(nblk)]
        ones_row = const.tile([1, 128], fp32)
        nc.vector.memset(ones_row[:], 1.0)
        ones_bf = const.tile([1, 128], bf16)
        nc.vector.memset(ones_bf[:], 1.0)

        def fm_copy(g_ap, pool, tag, dt):
            """(128,256) graph-major -> feature-major (128,2,128) via PE."""
            gf = pool.tile([128, 2, GBLK], dt, tag=tag)
            tp = ps_sm.tile([128, 2, 128], fp32, tag="pssm")
            for ki in range(2):
                nc.tensor.matmul(tp[:, ki, :], g_ap[:, ki * 128 : (ki + 1) * 128],
                                 eye[:], is_transpose=True, start=True, stop=True)
                nc.scalar.copy(gf[:, ki, :], tp[:, ki, :])
            return gf

        NH = NT // 2  # resident x split in halves so the next block's first
        # half can load while this block's second half is still in use

        def fused_logit(h1p_ap, scratch_ap, acc_ap):
            """accum(relu(h1) * sign-row); DVE only (GPSIMD can't read PSUM)."""
            nc.vector.scalar_tensor_tensor(
                scratch_ap, h1p_ap, 0.0, sgnbc[:],
                op0=mybir.AluOpType.max, op1=mybir.AluOpType.mult,
                accum_out=acc_ap,
            )

        for j in range(nblk):
            # bcols first: it unblocks all one-hot builds
            bt = bpool.tile([128, NT], fp32, tag="bcols")
            nc.sync.dma_start(bt[:], bcols_d[j])

            # ---- resident x (node-major bf16) for this block, two halves ----
            xhalves = []
            for h0 in (0, NH):
                xh = xres.tile([128, NH, H], bf16, tag="xres")
                for c0 in range(0, NH, LCHUNK):
                    cn = min(LCHUNK, NH - c0)
                    base = j * NTP + (h0 + c0) * 128
                    src = x_d[base : base + cn * 128, :].rearrange(
                        "(c p) h -> p c h", p=128
                    )
                    nc.sync.dma_start(xh[:, c0 : c0 + cn, :], src)
                xhalves.append(xh)

            def xat(t):
                return xhalves[t // NH][:, t % NH, :]

            # ---- resident x^T fp8 DoubleRow layout, two halves ----
            xthalves = []
            for h0 in (0, NH):
                xth = xtres.tile([128, NH, 2, 128], fp8, tag="xtres")
                for c0 in range(0, NH, LCHUNK):
                    cn = min(LCHUNK, NH - c0)
                    nc.sync.dma_start(
                        xth[:, c0 : c0 + cn, :, :],
                        xt_d[j, :, h0 + c0 : h0 + c0 + cn, :, :],
                    )
                xthalves.append(xth)

            def xtat(t):
                return xthalves[t // NH][:, t % NH, :, :]

            etj = etres.tile([128, NT, 128], bf16, tag="etres")

            # ---- phase A: initial mean pool + resident E^T build ----
            # e-builds run ahead (DVE); E^T via PE transpose + linear DMA
            # psum->sbuf; pool matmuls lag so the PE sees contiguous work.
            pooled = ps_pool.tile([GBLK, H], fp32, tag="pspool")
            ephA = []
            for t in range(NT + LAG):
                if t < NT:
                    e = epool.tile([128, 128], bf16, tag="eoh")
                    nc.vector.tensor_scalar(
                        e[:], iota_row[:], bt[:, t : t + 1], None,
                        op0=mybir.AluOpType.is_equal,
                    )
                    ephA.append(e)
                tl = t - LAG
                if tl >= 0:
                    e = ephA[tl]
                    nc.tensor.matmul(
                        pooled[:], e[:], xat(tl), start=(tl == 0),
                        stop=(tl == NT - 1), skip_group_check=True,
                    )
                    tpb = ps_sm.tile([128, 128], bf16, tag="pssm")
                    nc.tensor.matmul(tpb[:], e[:], eyeb[:], is_transpose=True,
                                     start=True, stop=True)
                    nc.scalar.copy(etj[:, tl, :], tpb[:])
            g_gm = gsb.tile([GBLK, H], fp32, tag="gsb")
            nc.vector.tensor_scalar(
                g_gm[:], pooled[:], invc[j][:], None, op0=mybir.AluOpType.mult
            )
            g_fm = fm_copy(g_gm[:], gsb, "gfm", bf16)

            # ---- timesteps ----
            for ts in range(NUM_TIMESTEPS):
                # GW1' = G @ W1'^T + b1'  (graph-level, bf16 -> fp8 copy)
                gw1p = ps_sm.tile([GBLK, H], fp32, tag="pssm")
                for ki in range(2):
                    nc.tensor.matmul(gw1p[:], g_fm[:, ki, :], w1tp[ki][:],
                                     start=(ki == 0), stop=False,
                                     skip_group_check=True)
                nc.tensor.matmul(gw1p[:], ones_bf[:], b1p[:],
                                 start=False, stop=True, skip_group_check=True)
                gw1bf = gsb.tile([GBLK, H], bf16, tag="gw1bf")
                nc.scalar.copy(gw1bf[:], gw1p[:])

                pooled = ps_pool.tile([GBLK, H], fp32, tag="pspool")

                # software-pipelined batches: h1 mms + fused logit run
                # ahead; sigmoid, eg builds and pool matmuls lag.
                h1ps = {}
                gacc = {}
                gt = {}

                def is_act_batch(b):
                    return (b % ACT_DEN) < ACT_NUM

                def emit_h1(b):
                    gaP = gtsb.tile([128, TB], fp32, tag="gacc", name="gaP")
                    gacc[b] = (gaP, None)
                    for c in range(TB):
                        t = b * TB + c
                        if c % 2 == 0:
                            h1pair = ps_h1.tile([128, 2, H], fp32, tag="psh1")
                        h1ps[t] = h1pair[:, c % 2, :]
                        nc.tensor.matmul(h1ps[t], xtat(t), w1dr[:],
                                         start=True, stop=False, perf_mode=DR)
                        nc.tensor.matmul(h1ps[t], etj[:, t, :], gw1bf[:],
                                         start=False, stop=True)

                def emit_fused(b):
                    gaP, _ = gacc[b]
                    act_path = is_act_batch(b)
                    for c in range(TB):
                        t = b * TB + c
                        if act_path:
                            # act drains the psum (plain relu -> bf16 sbuf),
                            # then DVE does the sign-dot at 2x on sbuf
                            rsb = scr.tile([128, H], bf16, tag="relu_sb")
                            nc.scalar.activation(
                                rsb[:], h1ps[t],
                                mybir.ActivationFunctionType.Relu,
                            )
                            sc = scr.tile([128, H], bf16, tag="scr")
                            nc.vector.scalar_tensor_tensor(
                                sc[:], rsb[:], 0.0, sgnbc[:],
                                op0=mybir.AluOpType.bypass,
                                op1=mybir.AluOpType.mult,
                                accum_out=gaP[:, c : c + 1],
                            )
                        else:
                            sc = scr.tile([128, H], bf16, tag="scr")
                            fused_logit(h1ps[t], sc[:], gaP[:, c : c + 1])
                        del h1ps[t]

                def emit_gate_pool(b):
                    gaP, gaN = gacc[b]
                    if gaN is not None:
                        gd = gtsb.tile([128, TB], fp32, tag="gacc")
                        nc.vector.tensor_sub(gd[:], gaP[:], gaN[:])
                        gaP = gd
                    g = gtsb.tile([128, TB], fp32, tag="gt")
                    gt[b] = g
                    nc.scalar.activation(
                        g[:], gaP[:],
                        mybir.ActivationFunctionType.Sigmoid,
                        bias=b2col[:], scale=float(2.0 ** (-S_SCALE)),
                    )
                    for c in range(TB):
                        t = b * TB + c
                        eg = epool.tile([128, 128], bf16, tag="eoh")
                        nc.vector.tensor_scalar(
                            eg[:], iota_row[:], bt[:, t : t + 1],
                            g[:, c : c + 1],
                            op0=mybir.AluOpType.is_equal,
                            op1=mybir.AluOpType.mult,
                        )
                        nc.tensor.matmul(
                            pooled[:], eg[:], xat(t),
                            start=(t == 0), stop=(t == NT - 1),
                            skip_group_check=True,
                        )

                for b in range(NB + LAG):
                    if b < NB:
                        emit_h1(b)
                    bl = b - LAG
                    if bl >= 0:
                        # gate+pool of the lagged batch BEFORE this batch's
                        # drains, so act's sigmoid isn't queued behind them
                        emit_gate_pool(bl)
                    if b < NB:
                        emit_fused(b)

                ps = gsb.tile([GBLK, H], fp32, tag="poolsb")
                nc.vector.tensor_scalar(
                    ps[:], pooled[:], invc[j][:], None,
                    op0=mybir.AluOpType.mult
                )
                pf = fm_copy(ps[:], gsb, "poolfm", bf16)

                # ---- GRU cell (graph-major) ----
                gf, h_old = g_fm, g_gm

                def gru_mm(psum, wi, wh, bias_row, bcol0, bn):
                    mms = []
                    if wi is not None:
                        mms += [(pf[:, ki, :], wi[ki][:, bcol0 : bcol0 + bn])
                                for ki in range(2)]
                    if wh is not None:
                        mms += [(gf[:, ki, :], wh[ki][:, bcol0 : bcol0 + bn])
                                for ki in range(2)]
                    for i, (lhsT, rhs) in enumerate(mms):
                        nc.tensor.matmul(
                            psum[:], lhsT, rhs, start=(i == 0), stop=False,
                            skip_group_check=True,
                        )
                    nc.tensor.matmul(
                        psum[:], ones_bf[:], bias_row, start=False, stop=True,
                        skip_group_check=True,
                    )

                rp = ps_h1.tile([GBLK, H], fp32, tag="psh1")
                gru_mm(rp, wih, whh, brz[:, 0:H], 0, H)
                r = smallsb.tile([GBLK, H], fp32, tag="gru_r")
                nc.scalar.activation(r[:], rp[:],
                                     mybir.ActivationFunctionType.Sigmoid)
                zp = ps_h1.tile([GBLK, H], fp32, tag="psh1")
                gru_mm(zp, wih, whh, brz[:, H : 2 * H], H, H)
                z = smallsb.tile([GBLK, H], fp32, tag="gru_z")
                nc.scalar.activation(z[:], zp[:],
                                     mybir.ActivationFunctionType.Sigmoid)
                inp_ = ps_h1.tile([GBLK, H], fp32, tag="psh1")
                gru_mm(inp_, wih, None, bin_[:], 2 * H, H)
                hnp = ps_h1.tile([GBLK, H], fp32, tag="psh1")
                gru_mm(hnp, None, whh, bhn[:], 2 * H, H)
                t1 = smallsb.tile([GBLK, H], fp32, tag="gru_s1")
                nc.vector.tensor_mul(t1[:], r[:], hnp[:])
                t2 = smallsb.tile([GBLK, H], fp32, tag="gru_s2")
                nc.vector.tensor_add(t2[:], t1[:], inp_[:])
                n = smallsb.tile([GBLK, H], fp32, tag="gru_n")
                nc.scalar.activation(n[:], t2[:],
                                     mybir.ActivationFunctionType.Tanh)
                t3 = smallsb.tile([GBLK, H], fp32, tag="gru_s1")
                nc.vector.tensor_sub(t3[:], h_old[:], n[:])
                t4 = smallsb.tile([GBLK, H], fp32, tag="gru_s2")
                nc.vector.tensor_mul(t4[:], z[:], t3[:])
                t5 = smallsb.tile([GBLK, H], fp32, tag="gru_s3")
                nc.vector.tensor_add(t5[:], n[:], t4[:])
                g_gm = gsb.tile([GBLK, H], fp32, tag="gsb")
                nc.scalar.activation(g_gm[:], t5[:],
                                     mybir.ActivationFunctionType.Relu)
                if ts < NUM_TIMESTEPS - 1:
                    g_fm = fm_copy(g_gm[:], gsb, "gfm", bf16)

            nc.sync.dma_start(out_d[j * GBLK : (j + 1) * GBLK, :], g_gm[:])

    nc.compile()
    return nc


def _prep_inputs(x, batch, counts, n_cores, nblk, NT=None):
    """Host-side shard + pad + layout. Returns (per_core, NT)."""
    import ml_dtypes

    G = n_cores * nblk * GBLK
    batch = np.asarray(batch).astype(np.int64)
    x = np.asarray(x, dtype=np.float32)

    edges = np.searchsorted(batch, np.arange(0, G + 1, GBLK))
    blk_cnt = np.diff(edges)
    if NT is None:
        NT = int(np.ceil(blk_cnt.max() / 128))
        NT = ((NT + TB - 1) // TB) * TB
        if (NT // TB) % 2:
            NT += TB  # halves must hold whole TB batches
    NTP = NT * 128

    invc_all = (1.0 / np.maximum(counts, 1.0)).astype(np.float32)

    xb = x.astype(ml_dtypes.bfloat16)
    x8 = x.astype(ml_dtypes.float8_e4m3)
    per_core = []
    for k in range(n_cores):
        xk = np.zeros((nblk * NTP, H), dtype=ml_dtypes.bfloat16)
        xk8 = np.zeros((nblk, NTP, H), dtype=ml_dtypes.float8_e4m3)
        bcols = np.full((nblk, 128, NT), -1.0, dtype=np.float32)
        for j in range(nblk):
            bi = k * nblk + j
            lo, hi = edges[bi], edges[bi + 1]
            cnt = hi - lo
            xk[j * NTP : j * NTP + cnt] = xb[lo:hi]
            xk8[j, :cnt] = x8[lo:hi]
            blp = np.full(NTP, -1.0, dtype=np.float32)
            blp[:cnt] = (batch[lo:hi] - (bi * GBLK)).astype(np.float32)
            bcols[j] = blp.reshape(NT, 128).T
        # [j, p(feat in ktile), t, k(ktile), n] = x8[j, t*128+n, k*128+p]
        xkT8 = np.ascontiguousarray(
            xk8.reshape(nblk, NT, 128, 2, 128).transpose(0, 4, 1, 3, 2)
        )
        invc = invc_all[k * nblk * GBLK : (k + 1) * nblk * GBLK].reshape(
            nblk, GBLK, 1
        )
        per_core.append({"xk": xk, "xkT8": xkT8, "bcols": bcols,
                         "invc": np.ascontiguousarray(invc)})
    return per_core, NT


def _const_inputs(gate_w1, gate_b1, gate_w2, gate_b2, gru_w_ih, gru_w_hh,
                  gru_b_ih, gru_b_hh):
    import ml_dtypes

    f = np.float32
    bf = ml_dtypes.bfloat16
    f8 = ml_dtypes.float8_e4m3
    c = {}
    w1 = np.asarray(gate_w1, f)  # (H, H), h1 = x @ w1.T
    w2 = np.asarray(gate_w2, f).reshape(H)
    # permute h1 columns so positive-w2 ones come first (sign-split drains)
    perm = np.concatenate([np.where(w2 >= 0.0)[0], np.where(w2 < 0.0)[0]])
    npos = int((w2 >= 0.0).sum())
    w1 = w1[perm]
    w2 = w2[perm]
    scale = (2.0 ** S_SCALE)
    w1p = w1 * (np.abs(w2)[:, None] * scale)  # fold |w2| into rows of W1
    b1p = np.asarray(gate_b1, f)[perm] * np.abs(w2) * scale
    sgn = np.where(w2 >= 0.0, 1.0, -1.0).astype(f)
    c["_npos"] = npos
    # w1dr[p, k, h] = w1p[h, k*128+p]
    c["w1dr"] = np.ascontiguousarray(
        w1p.T.reshape(2, 128, H).transpose(1, 0, 2)).astype(f8)
    c["w1tp"] = np.ascontiguousarray(w1p.T.reshape(2, 128, H)).astype(bf)
    c["b1prow"] = b1p.reshape(1, H).astype(bf)
    c["sgnbc"] = np.tile(sgn.reshape(1, H), (128, 1)).astype(bf)
    c["b2col"] = np.full((128, 1), np.asarray(gate_b2, f).reshape(()), dtype=f)
    c["wih_t"] = np.ascontiguousarray(
        np.asarray(gru_w_ih, f).T).reshape(2, 128, 3 * H).astype(bf)
    c["whh_t"] = np.ascontiguousarray(
        np.asarray(gru_w_hh, f).T).reshape(2, 128, 3 * H).astype(bf)
    bih = np.asarray(gru_b_ih, f)
    bhh = np.asarray(gru_b_hh, f)
    c["bsum_rz"] = (bih[: 2 * H] + bhh[: 2 * H]).reshape(1, 2 * H).astype(bf)
    c["bihn"] = bih[2 * H :].reshape(1, H).astype(bf)
    c["bhhn"] = bhh[2 * H :].reshape(1, H).astype(bf)
    c["iota_row"] = np.tile(np.arange(128, dtype=f), (128, 1)).astype(bf)
    c["eye128"] = np.eye(128, dtype=f)
    c["eye128b"] = np.eye(128, dtype=f).astype(bf)
    return c


_CACHE = {}


def run(x, gate_w1, gate_b1, gate_w2, gate_b2, gru_w_ih, gru_w_hh, gru_b_ih,
        gru_b_hh, batch, num_graphs, n_cores=8, nblk=NBLK, trace=False,
        use_sim=False):
    from concourse.bass_utils import run_bass_kernel_spmd

    batch = np.asarray(batch).astype(np.int64)
    G = n_cores * nblk * GBLK
    counts = np.bincount(batch, minlength=G).astype(np.float32)
    per_core, NT = _prep_inputs(x, batch, counts, n_cores, nblk)
    consts = _const_inputs(gate_w1, gate_b1, gate_w2, gate_b2, gru_w_ih,
                           gru_w_hh, gru_b_ih, gru_b_hh)
    npos = consts.pop("_npos")
    in_maps = [{**consts, **pc} for pc in per_core]

    key = (NT, npos, nblk, n_cores)
    if key not in _CACHE:
        _CACHE[key] = _build_program(NT, npos, nblk=nblk)
    nc = _CACHE[key]

    if use_sim:
        from concourse.bass_interp import CoreSim

        outs = []
        for k in range(n_cores):
            sim = CoreSim(nc)
            for name, arr in in_maps[k].items():
                sim.tensor(name)[:] = arr
            sim.simulate()
            outs.append(np.array(sim.tensor("out")))
        return np.concatenate(outs, axis=0), None

    res = run_bass_kernel_spmd(nc, in_maps, core_ids=list(range(n_cores)),
                               trace=trace)
    out = np.concatenate([res.results[k]["out"] for k in range(n_cores)], axis=0)
    return out, res


def kernel(**inputs):
    out, _ = run(**inputs)
    return out


# revision 52
# speedup vs baseline: 4.0625x; 1.0244x over previous
"""Trainium2 Bass kernel for AttentiveGraphPooling (gnn_message_passing).

Strategy: shard the 4096 graphs across 8 cores (512 graphs each). batch is
sorted, so each core owns a contiguous node range covering whole graphs ->
pooling / gather / GRU are all core-local, no collectives needed.

Per core, graphs go in 4 blocks of 128; nodes in NT tiles of 128 per block.
Resident per block: x node-major (bf16, for pooling), x feature-major in
fp8 DoubleRow layout (for the gate matmul), E^T one-hot (fp8, built once in
phase A via PE transpose).

Gate math: |w2| (and a 2^s scale) is folded into W1's columns host-side, so
  logit = sum_h sign(w2)_h * relu(h1'[n,h]),  h1' = x@W1'^T + GW1'[b(n)]
h1' is computed per node tile as ONE fp8 DoubleRow matmul (K=256) plus one
DoubleRow gather matmul (E^T tile paired with a zero k-tile). The whole
relu->*w2->reduce chain is ONE fused scalar_tensor_tensor (op0=max 0,
op1=mult sign-row, accum_out=logit column), split between the Vector and
GpSimd engines. Gates come from one batched sigmoid (scale=2^-s, bias=b2).

Weighted pooling stays bf16 (fp8 would lose too much precision in the
mean's cancellation): Eg one-hot built on DVE fused is_eq*gate, one matmul
per tile. Pool matmuls are emitted with a 2-batch lag so the PE runs long
uninterrupted bursts (p-state stays high).
"""

import os
import sys

import numpy as np

sys.path.insert(0, "/opt/trn_rl_repo")

H = 256
NBLK = 4  # graph blocks per core
GBLK = 128  # graphs per block
NUM_TIMESTEPS = 2
LCHUNK = 16  # node tiles per resident-load DMA
TB = 8  # gate batch (node tiles per sigmoid batch)
LAG = 2  # batches of lag for pool-matmul emission (PE burst length)
S_SCALE = 7  # 2^s fold into W1' so fp8 entries are in range

# drain-path schedule per 16 gate batches:
#   S = act sign-split relu+accum (tile leaves DVE entirely)
#   D = DVE psum-direct fused op
#   R = act relu drain + DVE sign-dot (two-stage)
DRAIN_PATTERN = "SDDRSDDSDDSDDSDD"


def _build_program(NT, npos, nblk=NBLK):
    """Build the single-core SPMD Bass program. NT = node tiles per block."""
    from contextlib import ExitStack

    import concourse.bass as bass
    import concourse.tile as tile
    from concourse import bacc, mybir

    fp32 = mybir.dt.float32
    bf16 = mybir.dt.bfloat16
    fp8 = mybir.dt.float8e4
    DR = mybir.MatmulPerfMode.DoubleRow

    NTP = NT * 128  # padded nodes per block
    NB = NT // TB  # gate batches per block

    nc = bacc.Bacc("TRN2", target_bir_lowering=False, debug=False)

    # ---- DRAM parameters (per-core inputs) ----
    assert 0 < npos < H
    x_d = nc.dram_tensor("xk", [nblk * NTP, H], bf16, kind="ExternalInput")
    xt_d = nc.dram_tensor("xkT8", [nblk, 128, NT, 2, 128], fp8, kind="ExternalInput")
    bcols_d = nc.dram_tensor("bcols", [nblk, 128, NT], fp32, kind="ExternalInput")
    invc_d = nc.dram_tensor("invc", [nblk, GBLK, 1], fp32, kind="ExternalInput")
    w1dr_d = nc.dram_tensor("w1dr", [128, 2, H], fp8, kind="ExternalInput")
    w1tp_d = nc.dram_tensor("w1tp", [2, 128, H], bf16, kind="ExternalInput")
    b1p_d = nc.dram_tensor("b1prow", [1, H], bf16, kind="ExternalInput")
    sgn_d = nc.dram_tensor("sgnbc", [128, H], bf16, kind="ExternalInput")
    b2c_d = nc.dram_tensor("b2col", [128, 1], fp32, kind="ExternalInput")
    wih_d = nc.dram_tensor("wih_t", [2, 128, 3 * H], bf16, kind="ExternalInput")
    whh_d = nc.dram_tensor("whh_t", [2, 128, 3 * H], bf16, kind="ExternalInput")
    brz_d = nc.dram_tensor("bsum_rz", [1, 2 * H], bf16, kind="ExternalInput")
    bin_d = nc.dram_tensor("bihn", [1, H], bf16, kind="ExternalInput")
    bhn_d = nc.dram_tensor("bhhn", [1, H], bf16, kind="ExternalInput")
    iota_d = nc.dram_tensor("iota_row", [128, 128], bf16, kind="ExternalInput")
    eye_d = nc.dram_tensor("eye128", [128, 128], fp32, kind="ExternalInput")
    eyeb_d = nc.dram_tensor("eye128b", [128, 128], bf16, kind="ExternalInput")
    out_d = nc.dram_tensor("out", [nblk * GBLK, H], fp32, kind="ExternalOutput")

    with tile.TileContext(nc) as tc, ExitStack() as ctx:
        ep = ctx.enter_context  # shorthand

        const = ep(tc.tile_pool(name="const", bufs=1))
        xres = ep(tc.tile_pool(name="xres", bufs=2))
        xtres = ep(tc.tile_pool(name="xtres", bufs=2))
        etres = ep(tc.tile_pool(name="etres", bufs=2))
        bpool = ep(tc.tile_pool(name="bcols", bufs=2))
        epool = ep(tc.tile_pool(name="eoh", bufs=6))
        scr = ep(tc.tile_pool(name="scr", bufs=6))
        gtsb = ep(tc.tile_pool(name="gtsb", bufs=4))
        gsb = ep(tc.tile_pool(name="gsb", bufs=2))
        smallsb = ep(tc.tile_pool(name="smallsb", bufs=1))

        ps_pool = ep(tc.tile_pool(name="pspool", bufs=2, space="PSUM"))
        ps_h1 = ep(tc.tile_pool(name="psh1", bufs=4, space="PSUM"))
        ps_sm = ep(tc.tile_pool(name="pssm", bufs=2, space="PSUM"))

        # ---- load constants ----
        def cload(shape, src, tag, dt=fp32):
            t = const.tile(shape, dt, tag=tag)
            nc.sync.dma_start(t[:], src)
            return t

        iota_row = cload([128, 128], iota_d[:], "c_iota", bf16)
        eye = cload([128, 128], eye_d[:], "c_eye")
        eyeb = cload([128, 128], eyeb_d[:], "c_eyeb", bf16)
        w1dr = cload([128, 2, H], w1dr_d[:], "c_w1dr", fp8)
        w1tp = [cload([128, H], w1tp_d[k], f"c_w1tp{k}", bf16) for k in range(2)]
        b1p = cload([1, H], b1p_d[:], "c_b1p", bf16)
        sgnbc = cload([128, H], sgn_d[:], "c_sgn", bf16)
        b2col = cload([128, 1], b2c_d[:], "c_b2c")
        wih = [cload([128, 3 * H], wih_d[k], f"c_wih{k}", bf16) for k in range(2)]
        whh = [cload([128, 3 * H], whh_d[k], f"c_whh{k}", bf16) for k in range(2)]
        brz = cload([1, 2 * H], brz_d[:], "c_brz", bf16)
        bin_ = cload([1, H], bin_d[:], "c_bin", bf16)
        bhn = cload([1, H], bhn_d[:], "c_bhn", bf16)
        invc = [cload([GBLK, 1], invc_d[j], f"c_invc{j}") for j in range(nblk)]
        ones_row = const.tile([1, 128], fp32)
        nc.vector.memset(ones_row[:], 1.0)
        ones_bf = const.tile([1, 128], bf16)
        nc.vector.memset(ones_bf[:], 1.0)

        def fm_copy(g_ap, pool, tag, dt):
            """(128,256) graph-major -> feature-major (128,2,128) via PE."""
            gf = pool.tile([128, 2, GBLK], dt, tag=tag)
            tp = ps_sm.tile([128, 2, 128], fp32, tag="pssm")
            for ki in range(2):
                nc.tensor.matmul(tp[:, ki, :], g_ap[:, ki * 128 : (ki + 1) * 128],
                                 eye[:], is_transpose=True, start=True, stop=True)
                nc.scalar.copy(gf[:, ki, :], tp[:, ki, :])
            return gf

        NH = NT // 2  # resident x split in halves so the next block's first
        # half can load while this block's second half is still in use

        def fused_logit(h1p_ap, scratch_ap, acc_ap):
            """accum(relu(h1) * sign-row); DVE only (GPSIMD can't read PSUM)."""
            nc.vector.scalar_tensor_tensor(
                scratch_ap, h1p_ap, 0.0, sgnbc[:],
                op0=mybir.AluOpType.max, op1=mybir.AluOpType.mult,
                accum_out=acc_ap,
            )

        for j in range(nblk):
            # bcols first: it unblocks all one-hot builds
            bt = bpool.tile([128, NT], fp32, tag="bcols")
            nc.sync.dma_start(bt[:], bcols_d[j])

            # ---- resident x (node-major bf16) for this block, two halves ----
            xhalves = []
            for h0 in (0, NH):
                xh = xres.tile([128, NH, H], bf16, tag="xres")
                for c0 in range(0, NH, LCHUNK):
                    cn = min(LCHUNK, NH - c0)
                    base = j * NTP + (h0 + c0) * 128
                    src = x_d[base : base + cn * 128, :].rearrange(
                        "(c p) h -> p c h", p=128
                    )
                    nc.sync.dma_start(xh[:, c0 : c0 + cn, :], src)
                xhalves.append(xh)

            def xat(t):
                return xhalves[t // NH][:, t % NH, :]

            # ---- resident x^T fp8 DoubleRow layout, two halves ----
            xthalves = []
            for h0 in (0, NH):
                xth = xtres.tile([128, NH, 2, 128], fp8, tag="xtres")
                for c0 in range(0, NH, LCHUNK):
                    cn = min(LCHUNK, NH - c0)
                    nc.sync.dma_start(
                        xth[:, c0 : c0 + cn, :, :],
                        xt_d[j, :, h0 + c0 : h0 + c0 + cn, :, :],
                    )
                xthalves.append(xth)

            def xtat(t):
                return xthalves[t // NH][:, t % NH, :, :]

            etj = etres.tile([128, NT, 128], bf16, tag="etres")

            # ---- phase A: initial mean pool + resident E^T build ----
            # e-builds run ahead (DVE); E^T via PE transpose + linear DMA
            # psum->sbuf; pool matmuls lag so the PE sees contiguous work.
            pooled = ps_pool.tile([GBLK, H], fp32, tag="pspool")
            ephA = []
            for t in range(NT + LAG):
                if t < NT:
                    e = epool.tile([128, 128], bf16, tag="eoh")
                    nc.vector.tensor_scalar(
                        e[:], iota_row[:], bt[:, t : t + 1], None,
                        op0=mybir.AluOpType.is_equal,
                    )
                    ephA.append(e)
                tl = t - LAG
                if tl >= 0:
                    e = ephA[tl]
                    nc.tensor.matmul(
                        pooled[:], e[:], xat(tl), start=(tl == 0),
                        stop=(tl == NT - 1), skip_group_check=True,
                    )
                    tpb = ps_sm.tile([128, 128], bf16, tag="pssm")
                    nc.tensor.matmul(tpb[:], e[:], eyeb[:], is_transpose=True,
                                     start=True, stop=True)
                    nc.scalar.copy(etj[:, tl, :], tpb[:])
            g_gm = gsb.tile([GBLK, H], fp32, tag="gsb")
            nc.vector.tensor_scalar(
                g_gm[:], pooled[:], invc[j][:], None, op0=mybir.AluOpType.mult
            )
            g_fm = fm_copy(g_gm[:], gsb, "gfm", bf16)

            # ---- timesteps ----
            for ts in range(NUM_TIMESTEPS):
                # GW1' = G @ W1'^T + b1'  (graph-level, bf16 -> fp8 copy)
                gw1p = ps_sm.tile([GBLK, H], fp32, tag="pssm")
                for ki in range(2):
                    nc.tensor.matmul(gw1p[:], g_fm[:, ki, :], w1tp[ki][:],
                                     start=(ki == 0), stop=False,
                                     skip_group_check=True)
                nc.tensor.matmul(gw1p[:], ones_bf[:], b1p[:],
                                 start=False, stop=True, skip_group_check=True)
                gw1bf = gsb.tile([GBLK, H], bf16, tag="gw1bf")
                nc.scalar.copy(gw1bf[:], gw1p[:])

                pooled = ps_pool.tile([GBLK, H], fp32, tag="pspool")

                # software-pipelined batches: h1 mms + fused logit run
                # ahead; sigmoid, eg builds and pool matmuls lag.
                h1ps = {}
                gacc = {}
                gt = {}

                def drain_kind(b):
                    return DRAIN_PATTERN[b % len(DRAIN_PATTERN)]

                def emit_h1(b):
                    gaP = gtsb.tile([128, TB], fp32, tag="gacc", name="gaP")
                    gaN = (gtsb.tile([128, TB], fp32, tag="gaccN", name="gaN")
                           if drain_kind(b) == "S" else None)
                    gacc[b] = (gaP, gaN)
                    for c in range(TB):
                        t = b * TB + c
                        if c % 2 == 0:
                            h1pair = ps_h1.tile([128, 2, H], fp32, tag="psh1")
                        h1ps[t] = h1pair[:, c % 2, :]
                        nc.tensor.matmul(h1ps[t], xtat(t), w1dr[:],
                                         start=True, stop=False, perf_mode=DR)
                        nc.tensor.matmul(h1ps[t], etj[:, t, :], gw1bf[:],
                                         start=False, stop=True)

                def emit_fused(b):
                    gaP, gaN = gacc[b]
                    kind = drain_kind(b)
                    for c in range(TB):
                        t = b * TB + c
                        if kind == "S":
                            # act-only: sign-split relu accumulate
                            scp = scr.tile([128, npos], bf16, tag="scrp")
                            nc.scalar.activation(
                                scp[:], h1ps[t][:, :npos],
                                mybir.ActivationFunctionType.Relu,
                                accum_out=gaP[:, c : c + 1],
                            )
                            scn = scr.tile([128, H - npos], bf16, tag="scrn")
                            nc.scalar.activation(
                                scn[:], h1ps[t][:, npos:],
                                mybir.ActivationFunctionType.Relu,
                                accum_out=gaN[:, c : c + 1],
                            )
                        elif kind == "R":
                            # act relu drain, DVE sign-dot on sbuf
                            rsb = scr.tile([128, H], bf16, tag="relu_sb")
                            nc.scalar.activation(
                                rsb[:], h1ps[t],
                                mybir.ActivationFunctionType.Relu,
                            )
                            sc = scr.tile([128, H], bf16, tag="scr")
                            nc.vector.scalar_tensor_tensor(
                                sc[:], rsb[:], 0.0, sgnbc[:],
                                op0=mybir.AluOpType.bypass,
                                op1=mybir.AluOpType.mult,
                                accum_out=gaP[:, c : c + 1],
                            )
                        else:
                            sc = scr.tile([128, H], bf16, tag="scr")
                            fused_logit(h1ps[t], sc[:], gaP[:, c : c + 1])
                        del h1ps[t]

                def emit_gate_pool(b):
                    gaP, gaN = gacc[b]
                    if gaN is not None:
                        gd = gtsb.tile([128, TB], fp32, tag="gacc")
                        nc.vector.tensor_sub(gd[:], gaP[:], gaN[:])
                        gaP = gd
                    g = gtsb.tile([128, TB], fp32, tag="gt")
                    gt[b] = g
                    nc.scalar.activation(
                        g[:], gaP[:],
                        mybir.ActivationFunctionType.Sigmoid,
                        bias=b2col[:], scale=float(2.0 ** (-S_SCALE)),
                    )
                    for c in range(TB):
                        t = b * TB + c
                        eg = epool.tile([128, 128], bf16, tag="eoh")
                        nc.vector.tensor_scalar(
                            eg[:], iota_row[:], bt[:, t : t + 1],
                            g[:, c : c + 1],
                            op0=mybir.AluOpType.is_equal,
                            op1=mybir.AluOpType.mult,
                        )
                        nc.tensor.matmul(
                            pooled[:], eg[:], xat(t),
                            start=(t == 0), stop=(t == NT - 1),
                            skip_group_check=True,
                        )

                for b in range(NB + LAG):
                    if b < NB:
                        emit_h1(b)
                    bl = b - LAG
                    if bl >= 0:
                        # gate+pool of the lagged batch BEFORE this batch's
                        # drains, so act's sigmoid isn't queued behind them
                        emit_gate_pool(bl)
                    if b < NB:
                        emit_fused(b)

                ps = gsb.tile([GBLK, H], fp32, tag="poolsb")
                nc.vector.tensor_scalar(
                    ps[:], pooled[:], invc[j][:], None,
                    op0=mybir.AluOpType.mult
                )
                pf = fm_copy(ps[:], gsb, "poolfm", bf16)

                # ---- GRU cell (graph-major) ----
                gf, h_old = g_fm, g_gm

                def gru_mm(psum, wi, wh, bias_row, bcol0, bn):
                    mms = []
                    if wi is not None:
                        mms += [(pf[:, ki, :], wi[ki][:, bcol0 : bcol0 + bn])
                                for ki in range(2)]
                    if wh is not None:
                        mms += [(gf[:, ki, :], wh[ki][:, bcol0 : bcol0 + bn])
                                for ki in range(2)]
                    for i, (lhsT, rhs) in enumerate(mms):
                        nc.tensor.matmul(
                            psum[:], lhsT, rhs, start=(i == 0), stop=False,
                            skip_group_check=True,
                        )
                    nc.tensor.matmul(
                        psum[:], ones_bf[:], bias_row, start=False, stop=True,
                        skip_group_check=True,
                    )

                rp = ps_h1.tile([GBLK, H], fp32, tag="psh1")
                gru_mm(rp, wih, whh, brz[:, 0:H], 0, H)
                r = smallsb.tile([GBLK, H], fp32, tag="gru_r")
                nc.scalar.activation(r[:], rp[:],
                                     mybir.ActivationFunctionType.Sigmoid)
                zp = ps_h1.tile([GBLK, H], fp32, tag="psh1")
                gru_mm(zp, wih, whh, brz[:, H : 2 * H], H, H)
                z = smallsb.tile([GBLK, H], fp32, tag="gru_z")
                nc.scalar.activation(z[:], zp[:],
                                     mybir.ActivationFunctionType.Sigmoid)
                inp_ = ps_h1.tile([GBLK, H], fp32, tag="psh1")
                gru_mm(inp_, wih, None, bin_[:], 2 * H, H)
                hnp = ps_h1.tile([GBLK, H], fp32, tag="psh1")
                gru_mm(hnp, None, whh, bhn[:], 2 * H, H)
                t1 = smallsb.tile([GBLK, H], fp32, tag="gru_s1")
                nc.vector.tensor_mul(t1[:], r[:], hnp[:])
                t2 = smallsb.tile([GBLK, H], fp32, tag="gru_s2")
                nc.vector.tensor_add(t2[:], t1[:], inp_[:])
                n = smallsb.tile([GBLK, H], fp32, tag="gru_n")
                nc.scalar.activation(n[:], t2[:],
                                     mybir.ActivationFunctionType.Tanh)
                t3 = smallsb.tile([GBLK, H], fp32, tag="gru_s1")
                nc.vector.tensor_sub(t3[:], h_old[:], n[:])
                t4 = smallsb.tile([GBLK, H], fp32, tag="gru_s2")
                nc.vector.tensor_mul(t4[:], z[:], t3[:])
                t5 = smallsb.tile([GBLK, H], fp32, tag="gru_s3")
                nc.vector.tensor_add(t5[:], n[:], t4[:])
                g_gm = gsb.tile([GBLK, H], fp32, tag="gsb")
                nc.scalar.activation(g_gm[:], t5[:],
                                     mybir.ActivationFunctionType.Relu)
                if ts < NUM_TIMESTEPS - 1:
                    g_fm = fm_copy(g_gm[:], gsb, "gfm", bf16)

            nc.sync.dma_start(out_d[j * GBLK : (j + 1) * GBLK, :], g_gm[:])

    nc.compile()
    return nc


def _prep_inputs(x, batch, counts, n_cores, nblk, NT=None):
    """Host-side shard + pad + layout. Returns (per_core, NT)."""
    import ml_dtypes

    G = n_cores * nblk * GBLK
    batch = np.asarray(batch).astype(np.int64)
    x = np.asarray(x, dtype=np.float32)

    edges = np.searchsorted(batch, np.arange(0, G + 1, GBLK))
    blk_cnt = np.diff(edges)
    if NT is None:
        NT = int(np.ceil(blk_cnt.max() / 128))
        NT = ((NT + TB - 1) // TB) * TB
        if (NT // TB) % 2:
            NT += TB  # halves must hold whole TB batches
    NTP = NT * 128

    invc_all = (1.0 / np.maximum(counts, 1.0)).astype(np.float32)

    xb = x.astype(ml_dtypes.bfloat16)
    x8 = x.astype(ml_dtypes.float8_e4m3)
    per_core = []
    for k in range(n_cores):
        xk = np.zeros((nblk * NTP, H), dtype=ml_dtypes.bfloat16)
        xk8 = np.zeros((nblk, NTP, H), dtype=ml_dtypes.float8_e4m3)
        bcols = np.full((nblk, 128, NT), -1.0, dtype=np.float32)
        for j in range(nblk):
            bi = k * nblk + j
            lo, hi = edges[bi], edges[bi + 1]
            cnt = hi - lo
            xk[j * NTP : j * NTP + cnt] = xb[lo:hi]
            xk8[j, :cnt] = x8[lo:hi]
            blp = np.full(NTP, -1.0, dtype=np.float32)
            blp[:cnt] = (batch[lo:hi] - (bi * GBLK)).astype(np.float32)
            bcols[j] = blp.reshape(NT, 128).T
        # [j, p(feat in ktile), t, k(ktile), n] = x8[j, t*128+n, k*128+p]
        xkT8 = np.ascontiguousarray(
            xk8.reshape(nblk, NT, 128, 2, 128).transpose(0, 4, 1, 3, 2)
        )
        invc = invc_all[k * nblk * GBLK : (k + 1) * nblk * GBLK].reshape(
            nblk, GBLK, 1
        )
        per_core.append({"xk": xk, "xkT8": xkT8, "bcols": bcols,
                         "invc": np.ascontiguousarray(invc)})
    return per_core, NT


def _const_inputs(gate_w1, gate_b1, gate_w2, gate_b2, gru_w_ih, gru_w_hh,
                  gru_b_ih, gru_b_hh):
    import ml_dtypes

    f = np.float32
    bf = ml_dtypes.bfloat16
    f8 = ml_dtypes.float8_e4m3
    c = {}
    w1 = np.asarray(gate_w1, f)  # (H, H), h1 = x @ w1.T
    w2 = np.asarray(gate_w2, f).reshape(H)
    # permute h1 columns so positive-w2 ones come first (sign-split drains)
    perm = np.concatenate([np.where(w2 >= 0.0)[0], np.where(w2 < 0.0)[0]])
    npos = int((w2 >= 0.0).sum())
    w1 = w1[perm]
    w2 = w2[perm]
    scale = (2.0 ** S_SCALE)
    w1p = w1 * (np.abs(w2)[:, None] * scale)  # fold |w2| into rows of W1
    b1p = np.asarray(gate_b1, f)[perm] * np.abs(w2) * scale
    sgn = np.where(w2 >= 0.0, 1.0, -1.0).astype(f)
    c["_npos"] = npos
    # w1dr[p, k, h] = w1p[h, k*128+p]
    c["w1dr"] = np.ascontiguousarray(
        w1p.T.reshape(2, 128, H).transpose(1, 0, 2)).astype(f8)
    c["w1tp"] = np.ascontiguousarray(w1p.T.reshape(2, 128, H)).astype(bf)
    c["b1prow"] = b1p.reshape(1, H).astype(bf)
    c["sgnbc"] = np.tile(sgn.reshape(1, H), (128, 1)).astype(bf)
    c["b2col"] = np.full((128, 1), np.asarray(gate_b2, f).reshape(()), dtype=f)
    c["wih_t"] = np.ascontiguousarray(
        np.asarray(gru_w_ih, f).T).reshape(2, 128, 3 * H).astype(bf)
    c["whh_t"] = np.ascontiguousarray(
        np.asarray(gru_w_hh, f).T).reshape(2, 128, 3 * H).astype(bf)
    bih = np.asarray(gru_b_ih, f)
    bhh = np.asarray(gru_b_hh, f)
    c["bsum_rz"] = (bih[: 2 * H] + bhh[: 2 * H]).reshape(1, 2 * H).astype(bf)
    c["bihn"] = bih[2 * H :].reshape(1, H).astype(bf)
    c["bhhn"] = bhh[2 * H :].reshape(1, H).astype(bf)
    c["iota_row"] = np.tile(np.arange(128, dtype=f), (128, 1)).astype(bf)
    c["eye128"] = np.eye(128, dtype=f)
    c["eye128b"] = np.eye(128, dtype=f).astype(bf)
    return c


_CACHE = {}


def run(x, gate_w1, gate_b1, gate_w2, gate_b2, gru_w_ih, gru_w_hh, gru_b_ih,
        gru_b_hh, batch, num_graphs, n_cores=8, nblk=NBLK, trace=False,
        use_sim=False):
    from concourse.bass_utils import run_bass_kernel_spmd

    batch = np.asarray(batch).astype(np.int64)
    G = n_cores * nblk * GBLK
    counts = np.bincount(batch, minlength=G).astype(np.float32)
    per_core, NT = _prep_inputs(x, batch, counts, n_cores, nblk)
    consts = _const_inputs(gate_w1, gate_b1, gate_w2, gate_b2, gru_w_ih,
                           gru_w_hh, gru_b_ih, gru_b_hh)
    npos = consts.pop("_npos")
    in_maps = [{**consts, **pc} for pc in per_core]

    key = (NT, npos, nblk, n_cores)
    if key not in _CACHE:
        _CACHE[key] = _build_program(NT, npos, nblk=nblk)
    nc = _CACHE[key]

    if use_sim:
        from concourse.bass_interp import CoreSim

        outs = []
        for k in range(n_cores):
            sim = CoreSim(nc)
            for name, arr in in_maps[k].items():
                sim.tensor(name)[:] = arr
            sim.simulate()
            outs.append(np.array(sim.tensor("out")))
        return np.concatenate(outs, axis=0), None

    res = run_bass_kernel_spmd(nc, in_maps, core_ids=list(range(n_cores)),
                               trace=trace)
    out = np.concatenate([res.results[k]["out"] for k in range(n_cores)], axis=0)
    return out, res


def kernel(**inputs):
    out, _ = run(**inputs)
    return out


# revision 53
# speedup vs baseline: 4.2771x; 1.0528x over previous
"""Trainium2 Bass kernel for AttentiveGraphPooling (gnn_message_passing).

Strategy: shard the 4096 graphs across 8 cores (512 graphs each). batch is
sorted, so each core owns a contiguous node range covering whole graphs ->
pooling / gather / GRU are all core-local, no collectives needed.

Per core, graphs go in 4 blocks of 128; nodes in NT tiles of 128 per block.
Resident per block: x node-major (bf16, for pooling), x feature-major in
fp8 DoubleRow layout (for the gate matmul), E^T one-hot (fp8, built once in
phase A via PE transpose).

Gate math: |w2| (and a 2^s scale) is folded into W1's columns host-side, so
  logit = sum_h sign(w2)_h * relu(h1'[n,h]),  h1' = x@W1'^T + GW1'[b(n)]
h1' is computed per node tile as ONE fp8 DoubleRow matmul (K=256) plus one
DoubleRow gather matmul (E^T tile paired with a zero k-tile). The whole
relu->*w2->reduce chain is ONE fused scalar_tensor_tensor (op0=max 0,
op1=mult sign-row, accum_out=logit column), split between the Vector and
GpSimd engines. Gates come from one batched sigmoid (scale=2^-s, bias=b2).

Weighted pooling stays bf16 (fp8 would lose too much precision in the
mean's cancellation): Eg one-hot built on DVE fused is_eq*gate, one matmul
per tile. Pool matmuls are emitted with a 2-batch lag so the PE runs long
uninterrupted bursts (p-state stays high).
"""

import os
import sys

import numpy as np

sys.path.insert(0, "/opt/trn_rl_repo")

H = 256
NBLK = 4  # graph blocks per core
GBLK = 128  # graphs per block
NUM_TIMESTEPS = 2
LCHUNK = 16  # node tiles per resident-load DMA
TB = 8  # gate batch (node tiles per sigmoid batch)
LAG = 3  # batches of lag for pool-matmul emission (PE burst length)
S_SCALE = 7  # 2^s fold into W1' so fp8 entries are in range

# drain-path schedule per 16 gate batches:
#   S = act sign-split relu+accum (tile leaves DVE entirely)
#   D = DVE psum-direct fused op
#   R = act relu drain + DVE sign-dot (two-stage)
DRAIN_PATTERN = "SDDRSDDSDDSDDSDD"


def _build_program(NT, npos, nblk=NBLK):
    """Build the single-core SPMD Bass program. NT = node tiles per block."""
    from contextlib import ExitStack

    import concourse.bass as bass
    import concourse.tile as tile
    from concourse import bacc, mybir

    fp32 = mybir.dt.float32
    bf16 = mybir.dt.bfloat16
    fp8 = mybir.dt.float8e4
    DR = mybir.MatmulPerfMode.DoubleRow

    NTP = NT * 128  # padded nodes per block
    NB = NT // TB  # gate batches per block

    nc = bacc.Bacc("TRN2", target_bir_lowering=False, debug=False)

    # ---- DRAM parameters (per-core inputs) ----
    assert 0 < npos < H
    x_d = nc.dram_tensor("xk", [nblk * NTP, H], bf16, kind="ExternalInput")
    xt_d = nc.dram_tensor("xkT8", [nblk, 128, NT, 2, 128], fp8, kind="ExternalInput")
    bcols_d = nc.dram_tensor("bcols", [nblk, 128, NT], fp32, kind="ExternalInput")
    invc_d = nc.dram_tensor("invc", [nblk, GBLK, 1], fp32, kind="ExternalInput")
    w1dr_d = nc.dram_tensor("w1dr", [128, 2, H], fp8, kind="ExternalInput")
    w1tp_d = nc.dram_tensor("w1tp", [2, 128, H], bf16, kind="ExternalInput")
    b1p_d = nc.dram_tensor("b1prow", [1, H], bf16, kind="ExternalInput")
    sgn_d = nc.dram_tensor("sgnbc", [128, H], bf16, kind="ExternalInput")
    b2c_d = nc.dram_tensor("b2col", [128, 1], fp32, kind="ExternalInput")
    wih_d = nc.dram_tensor("wih_t", [2, 128, 3 * H], bf16, kind="ExternalInput")
    whh_d = nc.dram_tensor("whh_t", [2, 128, 3 * H], bf16, kind="ExternalInput")
    brz_d = nc.dram_tensor("bsum_rz", [1, 2 * H], bf16, kind="ExternalInput")
    bin_d = nc.dram_tensor("bihn", [1, H], bf16, kind="ExternalInput")
    bhn_d = nc.dram_tensor("bhhn", [1, H], bf16, kind="ExternalInput")
    iota_d = nc.dram_tensor("iota_row", [128, 128], bf16, kind="ExternalInput")
    eye_d = nc.dram_tensor("eye128", [128, 128], fp32, kind="ExternalInput")
    eyeb_d = nc.dram_tensor("eye128b", [128, 128], bf16, kind="ExternalInput")
    out_d = nc.dram_tensor("out", [nblk * GBLK, H], fp32, kind="ExternalOutput")

    with tile.TileContext(nc) as tc, ExitStack() as ctx:
        ep = ctx.enter_context  # shorthand

        const = ep(tc.tile_pool(name="const", bufs=1))
        xres = ep(tc.tile_pool(name="xres", bufs=2))
        xtres = ep(tc.tile_pool(name="xtres", bufs=2))
        etres = ep(tc.tile_pool(name="etres", bufs=2))
        bpool = ep(tc.tile_pool(name="bcols", bufs=2))
        epool = ep(tc.tile_pool(name="eoh", bufs=8))
        scr = ep(tc.tile_pool(name="scr", bufs=6))
        gtsb = ep(tc.tile_pool(name="gtsb", bufs=6))
        gsb = ep(tc.tile_pool(name="gsb", bufs=2))
        smallsb = ep(tc.tile_pool(name="smallsb", bufs=1))

        ps_pool = ep(tc.tile_pool(name="pspool", bufs=2, space="PSUM"))
        ps_h1 = ep(tc.tile_pool(name="psh1", bufs=4, space="PSUM"))
        ps_sm = ep(tc.tile_pool(name="pssm", bufs=2, space="PSUM"))

        # ---- load constants ----
        def cload(shape, src, tag, dt=fp32):
            t = const.tile(shape, dt, tag=tag)
            nc.sync.dma_start(t[:], src)
            return t

        iota_row = cload([128, 128], iota_d[:], "c_iota", bf16)
        eye = cload([128, 128], eye_d[:], "c_eye")
        eyeb = cload([128, 128], eyeb_d[:], "c_eyeb", bf16)
        w1dr = cload([128, 2, H], w1dr_d[:], "c_w1dr", fp8)
        w1tp = [cload([128, H], w1tp_d[k], f"c_w1tp{k}", bf16) for k in range(2)]
        b1p = cload([1, H], b1p_d[:], "c_b1p", bf16)
        sgnbc = cload([128, H], sgn_d[:], "c_sgn", bf16)
        b2col = cload([128, 1], b2c_d[:], "c_b2c")
        wih = [cload([128, 3 * H], wih_d[k], f"c_wih{k}", bf16) for k in range(2)]
        whh = [cload([128, 3 * H], whh_d[k], f"c_whh{k}", bf16) for k in range(2)]
        brz = cload([1, 2 * H], brz_d[:], "c_brz", bf16)
        bin_ = cload([1, H], bin_d[:], "c_bin", bf16)
        bhn = cload([1, H], bhn_d[:], "c_bhn", bf16)
        invc = [cload([GBLK, 1], invc_d[j], f"c_invc{j}") for j in range(nblk)]
        ones_row = const.tile([1, 128], fp32)
        nc.vector.memset(ones_row[:], 1.0)
        ones_bf = const.tile([1, 128], bf16)
        nc.vector.memset(ones_bf[:], 1.0)

        def fm_copy(g_ap, pool, tag, dt):
            """(128,256) graph-major -> feature-major (128,2,128) via PE."""
            gf = pool.tile([128, 2, GBLK], dt, tag=tag)
            tp = ps_sm.tile([128, 2, 128], fp32, tag="pssm")
            for ki in range(2):
                nc.tensor.matmul(tp[:, ki, :], g_ap[:, ki * 128 : (ki + 1) * 128],
                                 eye[:], is_transpose=True, start=True, stop=True)
                nc.scalar.copy(gf[:, ki, :], tp[:, ki, :])
            return gf

        NH = NT // 2  # resident x split in halves so the next block's first
        # half can load while this block's second half is still in use

        def fused_logit(h1p_ap, scratch_ap, acc_ap):
            """accum(relu(h1) * sign-row); DVE only (GPSIMD can't read PSUM)."""
            nc.vector.scalar_tensor_tensor(
                scratch_ap, h1p_ap, 0.0, sgnbc[:],
                op0=mybir.AluOpType.max, op1=mybir.AluOpType.mult,
                accum_out=acc_ap,
            )

        for j in range(nblk):
            # bcols first: it unblocks all one-hot builds
            bt = bpool.tile([128, NT], fp32, tag="bcols")
            nc.sync.dma_start(bt[:], bcols_d[j])

            # ---- resident x (node-major bf16) for this block, two halves ----
            xhalves = []
            for h0 in (0, NH):
                xh = xres.tile([128, NH, H], bf16, tag="xres")
                for c0 in range(0, NH, LCHUNK):
                    cn = min(LCHUNK, NH - c0)
                    base = j * NTP + (h0 + c0) * 128
                    src = x_d[base : base + cn * 128, :].rearrange(
                        "(c p) h -> p c h", p=128
                    )
                    nc.sync.dma_start(xh[:, c0 : c0 + cn, :], src)
                xhalves.append(xh)

            def xat(t):
                return xhalves[t // NH][:, t % NH, :]

            # ---- resident x^T fp8 DoubleRow layout, two halves ----
            xthalves = []
            for h0 in (0, NH):
                xth = xtres.tile([128, NH, 2, 128], fp8, tag="xtres")
                for c0 in range(0, NH, LCHUNK):
                    cn = min(LCHUNK, NH - c0)
                    nc.sync.dma_start(
                        xth[:, c0 : c0 + cn, :, :],
                        xt_d[j, :, h0 + c0 : h0 + c0 + cn, :, :],
                    )
                xthalves.append(xth)

            def xtat(t):
                return xthalves[t // NH][:, t % NH, :, :]

            etj = etres.tile([128, NT, 128], bf16, tag="etres")

            # ---- phase A: initial mean pool + resident E^T build ----
            # e-builds run ahead (DVE); E^T via PE transpose + linear DMA
            # psum->sbuf; pool matmuls lag so the PE sees contiguous work.
            pooled = ps_pool.tile([GBLK, H], fp32, tag="pspool")
            ephA = []
            for t in range(NT + LAG):
                if t < NT:
                    e = epool.tile([128, 128], bf16, tag="eoh")
                    nc.vector.tensor_scalar(
                        e[:], iota_row[:], bt[:, t : t + 1], None,
                        op0=mybir.AluOpType.is_equal,
                    )
                    ephA.append(e)
                tl = t - LAG
                if tl >= 0:
                    e = ephA[tl]
                    nc.tensor.matmul(
                        pooled[:], e[:], xat(tl), start=(tl == 0),
                        stop=(tl == NT - 1), skip_group_check=True,
                    )
                    tpb = ps_sm.tile([128, 128], bf16, tag="pssm")
                    nc.tensor.matmul(tpb[:], e[:], eyeb[:], is_transpose=True,
                                     start=True, stop=True)
                    nc.scalar.copy(etj[:, tl, :], tpb[:])
            g_gm = gsb.tile([GBLK, H], fp32, tag="gsb")
            nc.vector.tensor_scalar(
                g_gm[:], pooled[:], invc[j][:], None, op0=mybir.AluOpType.mult
            )
            g_fm = fm_copy(g_gm[:], gsb, "gfm", bf16)

            # ---- timesteps ----
            for ts in range(NUM_TIMESTEPS):
                # GW1' = G @ W1'^T + b1'  (graph-level, bf16 -> fp8 copy)
                gw1p = ps_sm.tile([GBLK, H], fp32, tag="pssm")
                for ki in range(2):
                    nc.tensor.matmul(gw1p[:], g_fm[:, ki, :], w1tp[ki][:],
                                     start=(ki == 0), stop=False,
                                     skip_group_check=True)
                nc.tensor.matmul(gw1p[:], ones_bf[:], b1p[:],
                                 start=False, stop=True, skip_group_check=True)
                gw1bf = gsb.tile([GBLK, H], bf16, tag="gw1bf")
                nc.scalar.copy(gw1bf[:], gw1p[:])

                pooled = ps_pool.tile([GBLK, H], fp32, tag="pspool")

                # software-pipelined batches: h1 mms + fused logit run
                # ahead; sigmoid, eg builds and pool matmuls lag.
                h1ps = {}
                gacc = {}
                gt = {}

                def drain_kind(b):
                    return DRAIN_PATTERN[b % len(DRAIN_PATTERN)]

                def emit_h1(b):
                    gaP = gtsb.tile([128, TB], fp32, tag="gacc", name="gaP")
                    gaN = (gtsb.tile([128, TB], fp32, tag="gaccN", name="gaN")
                           if drain_kind(b) == "S" else None)
                    gacc[b] = (gaP, gaN)
                    for c in range(TB):
                        t = b * TB + c
                        if c % 2 == 0:
                            h1pair = ps_h1.tile([128, 2, H], fp32, tag="psh1")
                        h1ps[t] = h1pair[:, c % 2, :]
                        nc.tensor.matmul(h1ps[t], xtat(t), w1dr[:],
                                         start=True, stop=False, perf_mode=DR)
                        nc.tensor.matmul(h1ps[t], etj[:, t, :], gw1bf[:],
                                         start=False, stop=True)

                def emit_fused(b):
                    gaP, gaN = gacc[b]
                    kind = drain_kind(b)
                    for c in range(TB):
                        t = b * TB + c
                        if kind == "S":
                            # act-only: sign-split relu accumulate
                            scp = scr.tile([128, npos], bf16, tag="scrp")
                            nc.scalar.activation(
                                scp[:], h1ps[t][:, :npos],
                                mybir.ActivationFunctionType.Relu,
                                accum_out=gaP[:, c : c + 1],
                            )
                            scn = scr.tile([128, H - npos], bf16, tag="scrn")
                            nc.scalar.activation(
                                scn[:], h1ps[t][:, npos:],
                                mybir.ActivationFunctionType.Relu,
                                accum_out=gaN[:, c : c + 1],
                            )
                        elif kind == "R":
                            # act relu drain, DVE sign-dot on sbuf
                            rsb = scr.tile([128, H], bf16, tag="relu_sb")
                            nc.scalar.activation(
                                rsb[:], h1ps[t],
                                mybir.ActivationFunctionType.Relu,
                            )
                            sc = scr.tile([128, H], bf16, tag="scr")
                            nc.vector.scalar_tensor_tensor(
                                sc[:], rsb[:], 0.0, sgnbc[:],
                                op0=mybir.AluOpType.bypass,
                                op1=mybir.AluOpType.mult,
                                accum_out=gaP[:, c : c + 1],
                            )
                        else:
                            sc = scr.tile([128, H], bf16, tag="scr")
                            fused_logit(h1ps[t], sc[:], gaP[:, c : c + 1])
                        del h1ps[t]

                def emit_gate_pool(b):
                    gaP, gaN = gacc[b]
                    if gaN is not None:
                        gd = gtsb.tile([128, TB], fp32, tag="gacc")
                        nc.vector.tensor_sub(gd[:], gaP[:], gaN[:])
                        gaP = gd
                    g = gtsb.tile([128, TB], fp32, tag="gt")
                    gt[b] = g
                    nc.scalar.activation(
                        g[:], gaP[:],
                        mybir.ActivationFunctionType.Sigmoid,
                        bias=b2col[:], scale=float(2.0 ** (-S_SCALE)),
                    )
                    for c in range(TB):
                        t = b * TB + c
                        eg = epool.tile([128, 128], bf16, tag="eoh")
                        nc.vector.tensor_scalar(
                            eg[:], iota_row[:], bt[:, t : t + 1],
                            g[:, c : c + 1],
                            op0=mybir.AluOpType.is_equal,
                            op1=mybir.AluOpType.mult,
                        )
                        nc.tensor.matmul(
                            pooled[:], eg[:], xat(t),
                            start=(t == 0), stop=(t == NT - 1),
                            skip_group_check=True,
                        )

                for b in range(NB + LAG):
                    if b < NB:
                        emit_h1(b)
                    bl = b - LAG
                    if bl >= 0:
                        # gate+pool of the lagged batch BEFORE this batch's
                        # drains, so act's sigmoid isn't queued behind them
                        emit_gate_pool(bl)
                    if b < NB:
                        emit_fused(b)

                ps = gsb.tile([GBLK, H], fp32, tag="poolsb")
                nc.vector.tensor_scalar(
                    ps[:], pooled[:], invc[j][:], None,
                    op0=mybir.AluOpType.mult
                )
                pf = fm_copy(ps[:], gsb, "poolfm", bf16)

                # ---- GRU cell (graph-major) ----
                gf, h_old = g_fm, g_gm

                def gru_mm(psum, wi, wh, bias_row, bcol0, bn):
                    mms = []
                    if wi is not None:
                        mms += [(pf[:, ki, :], wi[ki][:, bcol0 : bcol0 + bn])
                                for ki in range(2)]
                    if wh is not None:
                        mms += [(gf[:, ki, :], wh[ki][:, bcol0 : bcol0 + bn])
                                for ki in range(2)]
                    for i, (lhsT, rhs) in enumerate(mms):
                        nc.tensor.matmul(
                            psum[:], lhsT, rhs, start=(i == 0), stop=False,
                            skip_group_check=True,
                        )
                    nc.tensor.matmul(
                        psum[:], ones_bf[:], bias_row, start=False, stop=True,
                        skip_group_check=True,
                    )

                rp = ps_h1.tile([GBLK, H], fp32, tag="psh1")
                gru_mm(rp, wih, whh, brz[:, 0:H], 0, H)
                r = smallsb.tile([GBLK, H], fp32, tag="gru_r")
                nc.scalar.activation(r[:], rp[:],
                                     mybir.ActivationFunctionType.Sigmoid)
                zp = ps_h1.tile([GBLK, H], fp32, tag="psh1")
                gru_mm(zp, wih, whh, brz[:, H : 2 * H], H, H)
                z = smallsb.tile([GBLK, H], fp32, tag="gru_z")
                nc.scalar.activation(z[:], zp[:],
                                     mybir.ActivationFunctionType.Sigmoid)
                inp_ = ps_h1.tile([GBLK, H], fp32, tag="psh1")
                gru_mm(inp_, wih, None, bin_[:], 2 * H, H)
                hnp = ps_h1.tile([GBLK, H], fp32, tag="psh1")
                gru_mm(hnp, None, whh, bhn[:], 2 * H, H)
                t1 = smallsb.tile([GBLK, H], fp32, tag="gru_s1")
                nc.vector.tensor_mul(t1[:], r[:], hnp[:])
                t2 = smallsb.tile([GBLK, H], fp32, tag="gru_s2")
                nc.vector.tensor_add(t2[:], t1[:], inp_[:])
                n = smallsb.tile([GBLK, H], fp32, tag="gru_n")
                nc.scalar.activation(n[:], t2[:],
                                     mybir.ActivationFunctionType.Tanh)
                t3 = smallsb.tile([GBLK, H], fp32, tag="gru_s1")
                nc.vector.tensor_sub(t3[:], h_old[:], n[:])
                t4 = smallsb.tile([GBLK, H], fp32, tag="gru_s2")
                nc.vector.tensor_mul(t4[:], z[:], t3[:])
                t5 = smallsb.tile([GBLK, H], fp32, tag="gru_s3")
                nc.vector.tensor_add(t5[:], n[:], t4[:])
                g_gm = gsb.tile([GBLK, H], fp32, tag="gsb")
                nc.scalar.activation(g_gm[:], t5[:],
                                     mybir.ActivationFunctionType.Relu)
                if ts < NUM_TIMESTEPS - 1:
                    g_fm = fm_copy(g_gm[:], gsb, "gfm", bf16)

            nc.sync.dma_start(out_d[j * GBLK : (j + 1) * GBLK, :], g_gm[:])

    nc.compile()
    return nc


def _prep_inputs(x, batch, counts, n_cores, nblk, NT=None):
    """Host-side shard + pad + layout. Returns (per_core, NT)."""
    import ml_dtypes

    G = n_cores * nblk * GBLK
    batch = np.asarray(batch).astype(np.int64)
    x = np.asarray(x, dtype=np.float32)

    edges = np.searchsorted(batch, np.arange(0, G + 1, GBLK))
    blk_cnt = np.diff(edges)
    if NT is None:
        NT = int(np.ceil(blk_cnt.max() / 128))
        NT = ((NT + TB - 1) // TB) * TB
        if (NT // TB) % 2:
            NT += TB  # halves must hold whole TB batches
    NTP = NT * 128

    invc_all = (1.0 / np.maximum(counts, 1.0)).astype(np.float32)

    xb = x.astype(ml_dtypes.bfloat16)
    x8 = x.astype(ml_dtypes.float8_e4m3)
    per_core = []
    for k in range(n_cores):
        xk = np.zeros((nblk * NTP, H), dtype=ml_dtypes.bfloat16)
        xk8 = np.zeros((nblk, NTP, H), dtype=ml_dtypes.float8_e4m3)
        bcols = np.full((nblk, 128, NT), -1.0, dtype=np.float32)
        for j in range(nblk):
            bi = k * nblk + j
            lo, hi = edges[bi], edges[bi + 1]
            cnt = hi - lo
            xk[j * NTP : j * NTP + cnt] = xb[lo:hi]
            xk8[j, :cnt] = x8[lo:hi]
            blp = np.full(NTP, -1.0, dtype=np.float32)
            blp[:cnt] = (batch[lo:hi] - (bi * GBLK)).astype(np.float32)
            bcols[j] = blp.reshape(NT, 128).T
        # [j, p(feat in ktile), t, k(ktile), n] = x8[j, t*128+n, k*128+p]
        xkT8 = np.ascontiguousarray(
            xk8.reshape(nblk, NT, 128, 2, 128).transpose(0, 4, 1, 3, 2)
        )
        invc = invc_all[k * nblk * GBLK : (k + 1) * nblk * GBLK].reshape(
            nblk, GBLK, 1
        )
        per_core.append({"xk": xk, "xkT8": xkT8, "bcols": bcols,
                         "invc": np.ascontiguousarray(invc)})
    return per_core, NT


def _const_inputs(gate_w1, gate_b1, gate_w2, gate_b2, gru_w_ih, gru_w_hh,
                  gru_b_ih, gru_b_hh):
    import ml_dtypes

    f = np.float32
    bf = ml_dtypes.bfloat16
    f8 = ml_dtypes.float8_e4m3
    c = {}
    w1 = np.asarray(gate_w1, f)  # (H, H), h1 = x @ w1.T
    w2 = np.asarray(gate_w2, f).reshape(H)
    # permute h1 columns so positive-w2 ones come first (sign-split drains)
    perm = np.concatenate([np.where(w2 >= 0.0)[0], np.where(w2 < 0.0)[0]])
    npos = int((w2 >= 0.0).sum())
    w1 = w1[perm]
    w2 = w2[perm]
    scale = (2.0 ** S_SCALE)
    w1p = w1 * (np.abs(w2)[:, None] * scale)  # fold |w2| into rows of W1
    b1p = np.asarray(gate_b1, f)[perm] * np.abs(w2) * scale
    sgn = np.where(w2 >= 0.0, 1.0, -1.0).astype(f)
    c["_npos"] = npos
    # w1dr[p, k, h] = w1p[h, k*128+p]
    c["w1dr"] = np.ascontiguousarray(
        w1p.T.reshape(2, 128, H).transpose(1, 0, 2)).astype(f8)
    c["w1tp"] = np.ascontiguousarray(w1p.T.reshape(2, 128, H)).astype(bf)
    c["b1prow"] = b1p.reshape(1, H).astype(bf)
    c["sgnbc"] = np.tile(sgn.reshape(1, H), (128, 1)).astype(bf)
    c["b2col"] = np.full((128, 1), np.asarray(gate_b2, f).reshape(()), dtype=f)
    c["wih_t"] = np.ascontiguousarray(
        np.asarray(gru_w_ih, f).T).reshape(2, 128, 3 * H).astype(bf)
    c["whh_t"] = np.ascontiguousarray(
        np.asarray(gru_w_hh, f).T).reshape(2, 128, 3 * H).astype(bf)
    bih = np.asarray(gru_b_ih, f)
    bhh = np.asarray(gru_b_hh, f)
    c["bsum_rz"] = (bih[: 2 * H] + bhh[: 2 * H]).reshape(1, 2 * H).astype(bf)
    c["bihn"] = bih[2 * H :].reshape(1, H).astype(bf)
    c["bhhn"] = bhh[2 * H :].reshape(1, H).astype(bf)
    c["iota_row"] = np.tile(np.arange(128, dtype=f), (128, 1)).astype(bf)
    c["eye128"] = np.eye(128, dtype=f)
    c["eye128b"] = np.eye(128, dtype=f).astype(bf)
    return c


_CACHE = {}


def run(x, gate_w1, gate_b1, gate_w2, gate_b2, gru_w_ih, gru_w_hh, gru_b_ih,
        gru_b_hh, batch, num_graphs, n_cores=8, nblk=NBLK, trace=False,
        use_sim=False):
    from concourse.bass_utils import run_bass_kernel_spmd

    batch = np.asarray(batch).astype(np.int64)
    G = n_cores * nblk * GBLK
    counts = np.bincount(batch, minlength=G).astype(np.float32)
    per_core, NT = _prep_inputs(x, batch, counts, n_cores, nblk)
    consts = _const_inputs(gate_w1, gate_b1, gate_w2, gate_b2, gru_w_ih,
                           gru_w_hh, gru_b_ih, gru_b_hh)
    npos = consts.pop("_npos")
    in_maps = [{**consts, **pc} for pc in per_core]

    key = (NT, npos, nblk, n_cores)
    if key not in _CACHE:
        _CACHE[key] = _build_program(NT, npos, nblk=nblk)
    nc = _CACHE[key]

    if use_sim:
        from concourse.bass_interp import CoreSim

        outs = []
        for k in range(n_cores):
            sim = CoreSim(nc)
            for name, arr in in_maps[k].items():
                sim.tensor(name)[:] = arr
            sim.simulate()
            outs.append(np.array(sim.tensor("out")))
        return np.concatenate(outs, axis=0), None

    res = run_bass_kernel_spmd(nc, in_maps, core_ids=list(range(n_cores)),
                               trace=trace)
    out = np.concatenate([res.results[k]["out"] for k in range(n_cores)], axis=0)
    return out, res


def kernel(**inputs):
    out, _ = run(**inputs)
    return out
